# revision 1
# baseline (speedup 1.0000x reference)
"""Trainium2 Bass kernel for DenseFeatureExtractionModule (irregular-pooled VGG).

Sharding: 8 cores = 2 images x 4 row-strips of the 192-grid output (48 rows
each). Each core receives its input strip with enough halo rows to compute
all 10 conv layers locally (no inter-core communication). Out-of-image halo
rows are kept at zero through the layer stack by multiplying edge-band rows
with a per-core row-validity mask, which reproduces SAME-conv zero padding.

Per-layer structure on device: activations live in DRAM scratch buffers laid
out [C, rows, W+2*PAD] with zeroed pad columns; 3x3 convs are 9 shifted
matmuls (K=C_in tile, N=2 rows x 192 cols) accumulated in PSUM, with
float32r operands for full-rate fp32 MACs on the PE. Graph-conv layers
compute both the dilation-1 and dilation-s results and per-pixel select with
the (host-upsampled) pooling mask via copy_predicated. The two irregular
pool stages (block-max + replicate, masked) are fused into the producing
layer's output path.
"""

import numpy as np

import concourse.bacc as bacc
import concourse.bass as bass
import concourse.mybir as mybir
import concourse.tile as tile
from concourse.bass_utils import run_bass_kernel_spmd

F32 = mybir.dt.float32
F32R = mybir.dt.float32r
U8 = mybir.dt.uint8
RELU = mybir.ActivationFunctionType.Relu
MAX = mybir.AluOpType.max

W192 = 192
W384 = 384
PAD = 4  # pad columns for all 192-grid buffers
WP = W192 + 2 * PAD  # 200
A1_WP = W384 + 2  # 386, pad 1

# buffer row counts (per core strips, incl. halo)
CANVAS_ROWS = 180  # batch strip on 384 grid (96 + 2*42)
A1_ROWS = 178
ROWS192 = {"A2": 88, "A3": 86, "A4": 84, "A5": 80, "A6": 76, "A7": 72,
           "A8": 64, "A9": 56, "OUT": 48}
HALO192 = {"A2": 20, "A3": 19, "A4": 18, "A5": 16, "A6": 14, "A7": 12,
           "A8": 8, "A9": 4, "OUT": 0}

_CHANS = [(3, 64), (64, 64), (64, 128), (128, 128), (128, 256),
          (256, 256), (256, 256), (256, 512), (512, 512), (512, 512)]

# (src, dst, Cin, Cout, dils, pool_s, widx) for layers 3..10
LAYERS = [
    ("A2", "A3", 64, 128, (1,), None, 3),
    ("A3", "A4", 128, 128, (1,), 2, 4),
    ("A4", "A5", 128, 256, (1, 2), None, 5),
    ("A5", "A6", 256, 256, (1, 2), None, 6),
    ("A6", "A7", 256, 256, (1, 2), 4, 7),
    ("A7", "A8", 256, 512, (1, 4), None, 8),
    ("A8", "A9", 512, 512, (1, 4), None, 9),
    ("A9", "OUT", 512, 512, (1, 4), None, 10),
]
STREAM_W = {9, 10}  # layers whose weights are streamed per (block, co-tile)
RB = 8  # output rows per input-tile block (192-grid layers)

TAPS = [(a, b) for a in (-1, 0, 1) for b in (-1, 0, 1)]


def _f32v(x):
    return x.bitcast(F32) if x.dtype == F32R else x


def _r32(x):
    return x


def _emit_zero_pads(nc, tc, pools, bufs):
    """Zero the pad columns of every 192-grid DRAM buffer (and A1)."""
    zt = pools["const"].tile([128, ROWS192["A2"] * PAD], F32R)
    nc.gpsimd.memset(zt[:].bitcast(F32), 0.0)
    for name, dram in bufs.items():
        if name in ("X1", "OUT"):
            continue
        C, rows, wp = dram.shape
        pad = 1 if name == "A1" else PAD
        w = wp - 2 * pad
        for ci in range((C + 127) // 128):
            p = min(128, C - ci * 128)
            src = zt[:p, : rows * pad].rearrange("p (r c) -> p r c", c=pad)
            nc.sync.dma_start(dram[ci * 128 : ci * 128 + p, :, 0:pad], src)
            nc.sync.dma_start(dram[ci * 128 : ci * 128 + p, :, wp - pad : wp], src)


def _emit_l1(nc, tc, pools, bufs, params):
    """L1: 1x1 conv over im2col'd input (K=27, M=64), 384 grid."""
    from contextlib import ExitStack
    x1, a1 = bufs["X1"], bufs["A1"]
    wsb = pools["const"].tile([27, 64], F32R)
    nc.sync.dma_start(wsb[:], params["w1"][:])
    bsb = pools["const"].tile([64, 1], F32)
    nc.sync.dma_start(bsb[:], params["b1"][:])
    rm = pools["const"].tile([128, CANVAS_ROWS], F32)
    nc.sync.dma_start(rm[:], params["rm384"][:])

    RB1 = 16
    with tc.tile_pool(name="in1", bufs=2) as p_in1, \
         tc.tile_pool(name="out1", bufs=4) as p_out1:
     for i0 in range(0, A1_ROWS, RB1):
        nr = min(RB1, A1_ROWS - i0)
        xt = p_in1.tile([27, RB1, W384], F32R, tag="x1t")
        nc.sync.dma_start(xt[:, :nr, :], x1[:, i0 : i0 + nr, :])
        for j in range(nr):
            i = i0 + j
            ps = pools["psum"].tile([64, W384], F32, tag="ps")
            nc.tensor.matmul(ps[:], _r32(wsb[:]), _r32(xt[:, j, :]),
                             start=True, stop=True)
            ot = p_out1.tile([64, W384], F32R, tag="o1")
            nc.scalar.activation(ot[:], ps[:], RELU, bias=bsb[:])
            if i < 41 or i >= A1_ROWS - 41:
                nc.vector.tensor_mul(
                    ot[:], ot[:],
                    rm[0:64, i + 1 : i + 2].broadcast_to([64, W384]))
            nc.sync.dma_start(a1[:, i, 1 : 1 + W384], ot[:])


def _emit_l2(nc, tc, pools, bufs, params):
    """L2 conv (64->64, 384 grid) fused with 2x2 maxpool -> A2 (192 grid)."""
    a1, a2 = bufs["A1"], bufs["A2"]
    wsb = pools["const"].tile([64, 9, 64], F32R)
    nc.sync.dma_start(wsb[:], params["w2"][:])
    bsb = pools["const"].tile([64, 1], F32)
    nc.sync.dma_start(bsb[:], params["b2"][:])
    rm = pools["rm192"]

    QB = 8  # A2 rows per block
    with tc.tile_pool(name="in2", bufs=2) as p_in2, \
         tc.tile_pool(name="out2", bufs=4) as p_out2:
     for q0 in range(0, ROWS192["A2"], QB):
        nq = min(QB, ROWS192["A2"] - q0)
        xt = p_in2.tile([64, 2 * QB + 2, A1_WP], F32R, tag="x2t")
        nc.sync.dma_start(xt[:, : 2 * nq + 2, :],
                          a1[:, 2 * q0 : 2 * q0 + 2 * nq + 2, :])
        for q in range(q0, q0 + nq):
            o2 = p_out2.tile([64, 2, W384], F32R, tag="o2")
            for r in range(2):
                ps = pools["psum"].tile([64, W384], F32, tag="ps")
                for ti, (a, b) in enumerate(TAPS):
                    rhs = xt[:, 2 * (q - q0) + r + 1 + a, 1 + b : 1 + b + W384]
                    nc.tensor.matmul(ps[:], _r32(wsb[:, ti, :]), _r32(rhs),
                                     start=(ti == 0), stop=(ti == 8))
                nc.scalar.activation(o2[:, r, :], ps[:], RELU, bias=bsb[:])
            o2v = o2[:].rearrange("p r (c t) -> p r c t", t=2)
            cm = p_out2.tile([64, 2, W192], F32R, tag="cm2")
            nc.vector.tensor_tensor(cm[:, 0, :], o2v[:, 0, :, 0], o2v[:, 0, :, 1], MAX)
            nc.vector.tensor_tensor(cm[:, 1, :], o2v[:, 1, :, 0], o2v[:, 1, :, 1], MAX)
            mp = p_out2.tile([64, W192], F32R, tag="mp2")
            nc.vector.tensor_tensor(mp[:], cm[:, 0, :], cm[:, 1, :], MAX)
            if q < 20 or q >= ROWS192["A2"] - 20:
                nc.vector.tensor_mul(
                    mp[:], mp[:], rm[0:64, q : q + 1].broadcast_to([64, W192]))
            nc.sync.dma_start(a2[:, q, PAD : PAD + W192], mp[:])


def _emit_conv192(nc, tc, pools, bufs, params, src, dst, cin, cout, dils,
                  pool_s, widx):
    """Generic 192-grid conv layer with optional dual dilation + mask select
    and fused irregular pooling."""
    sdram, ddram = bufs[src], bufs[dst]
    rows_out = ROWS192[dst]
    h_out = HALO192[dst]
    dm = max(dils)
    nci = (cin + 127) // 128
    nco = (cout + 127) // 128
    off = 20 - h_out  # slice offset into A2-grid masks
    graph = len(dils) == 2
    act_dt = F32 if dst == "OUT" else F32R
    rm = pools["rm192"]
    msb = pools["m_u8"]
    dst_c0 = 0 if dst == "OUT" else PAD

    stream = widx in STREAM_W
    gpath = graph and widx >= 8
    rb_layer = 2 if gpath else (4 if cin > 256 else RB)
    if gpath:
        stream = False
    if not stream:
        wts = []
        for ci in range(nci):
            p = min(128, cin - ci * 128)
            wt = pools["wres"].tile([p, 9, cout], F32R, name=f"w{widx}_{ci}",
                                    tag=f"wres_{ci}")
            nc.sync.dma_start(wt[:], params[f"w{widx}"][ci * 128 : ci * 128 + p])
            wts.append(wt)
    bsb = pools["const"].tile([min(cout, 128), nco], F32, name=f"bsb{widx}")
    nc.sync.dma_start(bsb[:], params[f"b{widx}"][:])

    if gpath:
        s2 = dils[1]
        with tc.tile_pool(name=f"xin{widx}", bufs=2) as p_xin, \
             tc.tile_pool(name=f"gp{widx}", bufs=4) as p_g:
            for j0 in range(0, rows_out, 2):
                xts = []
                for ci in range(nci):
                    p = min(128, cin - ci * 128)
                    xt = p_xin.tile([p, 2 + 2 * dm, WP], F32R, tag=f"xin{ci}")
                    nc.sync.dma_start(
                        xt[:], sdram[ci * 128 : ci * 128 + p,
                                     j0 : j0 + 2 + 2 * dm, :])
                    xts.append(xt)
                pss = [pools["psum"].tile([128, 2 * W192], F32, tag="ps",
                                          name=f"ps{widx}_{j0}_{i}")
                       for i in range(nco)]
                for ci in range(nci):
                    p = min(128, cin - ci * 128)
                    for ti, (a, b) in enumerate(TAPS):
                        g1 = p_g.tile([p, 2, W192], F32R, tag="g1")
                        g2 = p_g.tile([p, 2, W192], F32R, tag="g2")
                        nc.vector.tensor_copy(
                            _f32v(g1[:]),
                            _f32v(xts[ci][:, dm + a : dm + a + 2,
                                          PAD + b : PAD + b + W192]))
                        nc.vector.tensor_copy(
                            _f32v(g2[:]),
                            _f32v(xts[ci][:, dm + a * s2 : dm + a * s2 + 2,
                                          PAD + b * s2 : PAD + b * s2 + W192]))
                        nc.vector.copy_predicated(
                            _f32v(g1[:]), msb[:p, off + j0 : off + j0 + 2, :],
                            _f32v(g2[:]))
                        g3 = p_g.tile([p, 2, W192], F32R, tag="g3")
                        nc.vector.tensor_copy(g3[:], _f32v(g1[:]))
                        for co in range(nco):
                            pco = min(128, cout - co * 128)
                            nc.tensor.matmul(
                                pss[co][:pco, :],
                                wts[ci][:, ti, co * 128 : co * 128 + pco],
                                g3[:],
                                start=(ci == 0 and ti == 0),
                                stop=(ci == nci - 1 and ti == 8))
                for co in range(nco):
                    pco = min(128, cout - co * 128)
                    tg = pools["oacc"].tile([pco, 2, W192], act_dt, tag="oacc")
                    psv = pss[co][:pco, :].rearrange("p (r w) -> p r w", w=W192)
                    nc.scalar.activation(tg[:], psv, RELU,
                                         bias=bsb[:pco, co : co + 1])
                    if j0 < h_out or j0 + 2 > rows_out - h_out:
                        nc.vector.tensor_mul(
                            tg[:], tg[:],
                            rm[:pco, off + j0 : off + j0 + 2].unsqueeze(-1)
                            .broadcast_to([pco, 2, W192]))
                    nc.sync.dma_start(
                        ddram[co * 128 : co * 128 + pco, j0 : j0 + 2,
                              dst_c0 : dst_c0 + W192], tg[:])
        return

    grp = pool_s if pool_s else 2  # rows per output tile group
    with tc.tile_pool(name=f"xin{widx}", bufs=2) as p_xin:
     for j0 in range(0, rows_out, rb_layer):
        rb = min(rb_layer, rows_out - j0)
        xts = []
        for ci in range(nci):
            p = min(128, cin - ci * 128)
            xt = p_xin.tile([p, rb_layer + 2 * dm, WP], F32R, tag=f"xin{ci}")
            nc.sync.dma_start(xt[:, : rb + 2 * dm, :],
                              sdram[ci * 128 : ci * 128 + p, j0 : j0 + rb + 2 * dm, :])
            xts.append(xt)
        for co in range(nco):
            pco = min(128, cout - co * 128)
            if stream:
                wts = []
                for ci in range(nci):
                    p = min(128, cin - ci * 128)
                    wt = pools["wstr"].tile([p, 9, 128], F32R, tag=f"ws{ci}")
                    nc.sync.dma_start(
                        wt[:, :, :pco],
                        params[f"w{widx}"][ci * 128 : ci * 128 + p, :,
                                           co * 128 : co * 128 + pco])
                    wts.append(wt)
            for g0 in range(0, rb, grp):
                tg = pools["oacc"].tile([pco, grp, W192], act_dt, tag="oacc")
                for rp in range(grp // 2):
                    j = j0 + g0 + rp * 2
                    pss = []
                    for d in dils:
                        ps = pools["psum"].tile([pco, 2 * W192], F32, tag="ps")
                        for ci in range(nci):
                            for ti, (a, b) in enumerate(TAPS):
                                rhs = xts[ci][:, g0 + rp * 2 + dm + a * d :
                                              g0 + rp * 2 + dm + a * d + 2,
                                              PAD + b * d : PAD + b * d + W192]
                                nc.tensor.matmul(
                                    ps[:],
                                    _r32(wts[ci][:, ti, co * 128 : co * 128 + pco]
                                         if not stream else wts[ci][:, ti, :pco]),
                                    _r32(rhs),
                                    start=(ci == 0 and ti == 0),
                                    stop=(ci == nci - 1 and ti == 8))
                        pss.append(ps)
                    t1 = tg[:, rp * 2 : rp * 2 + 2, :]
                    psv = pss[0][:].rearrange("p (r w) -> p r w", w=W192)
                    nc.scalar.activation(t1, psv, RELU, bias=bsb[:pco, co : co + 1])
                    if graph:
                        t2 = pools["osel"].tile([pco, 2, W192], act_dt, tag="osel")
                        ps2v = pss[1][:].rearrange("p (r w) -> p r w", w=W192)
                        nc.scalar.activation(t2[:], ps2v, RELU,
                                             bias=bsb[:pco, co : co + 1])
                        nc.vector.copy_predicated(
                            _f32v(t1), msb[:pco, off + j : off + j + 2, :],
                            _f32v(t2[:]))
                    if j < h_out or j + 2 > rows_out - h_out:
                        nc.vector.tensor_mul(
                            t1, t1,
                            rm[:pco, off + j : off + j + 2].unsqueeze(-1)
                            .broadcast_to([pco, 2, W192]))
                j = j0 + g0
                if pool_s == 2:
                    tv = tg[:].rearrange("p r (c t) -> p r c t", t=2)
                    cm = pools["pscr"].tile([pco, 2, W192 // 2], F32R, tag="pcm")
                    nc.vector.tensor_tensor(cm[:], tv[:, :, :, 0], tv[:, :, :, 1], MAX)
                    bm = pools["pscr"].tile([pco, W192 // 2], F32R, tag="pbm")
                    nc.vector.tensor_tensor(bm[:], cm[:, 0, :], cm[:, 1, :], MAX)
                    rep = pools["pscr"].tile([pco, 2, W192], F32R, tag="prep")
                    nc.vector.tensor_copy(
                        _f32v(rep[:]), _f32v(bm[:]).unsqueeze(1).unsqueeze(-1)
                        .broadcast_to([pco, 2, W192 // 2, 2]))
                    nc.vector.copy_predicated(
                        _f32v(tg[:]), msb[:pco, off + j : off + j + 2, :],
                        _f32v(rep[:]))
                elif pool_s == 4:
                    tv = tg[:].rearrange("p r (c t) -> p r c t", t=4)
                    c1 = pools["pscr"].tile([pco, 4, W192 // 4], F32R, tag="pc1")
                    c2 = pools["pscr"].tile([pco, 4, W192 // 4], F32R, tag="pc2")
                    nc.vector.tensor_tensor(c1[:], tv[:, :, :, 0], tv[:, :, :, 1], MAX)
                    nc.vector.tensor_tensor(c2[:], tv[:, :, :, 2], tv[:, :, :, 3], MAX)
                    nc.vector.tensor_tensor(c1[:], c1[:], c2[:], MAX)
                    r1 = pools["pscr"].tile([pco, W192 // 4], F32R, tag="pr1")
                    r2 = pools["pscr"].tile([pco, W192 // 4], F32R, tag="pr2")
                    nc.vector.tensor_tensor(r1[:], c1[:, 0, :], c1[:, 1, :], MAX)
                    nc.vector.tensor_tensor(r2[:], c1[:, 2, :], c1[:, 3, :], MAX)
                    nc.vector.tensor_tensor(r1[:], r1[:], r2[:], MAX)
                    rep = pools["pscr"].tile([pco, 4, W192], F32R, tag="prep4")
                    nc.vector.tensor_copy(
                        _f32v(rep[:]), _f32v(r1[:]).unsqueeze(1).unsqueeze(-1)
                        .broadcast_to([pco, 4, W192 // 4, 4]))
                    nc.vector.copy_predicated(
                        _f32v(tg[:]), msb[:pco, off + j : off + j + 4, :],
                        _f32v(rep[:]))
                nc.sync.dma_start(
                    ddram[co * 128 : co * 128 + pco, j : j + grp,
                          dst_c0 : dst_c0 + W192], tg[:])


def build_program():
    nc = bacc.Bacc()
    params = {}
    params["x1col"] = nc.declare_dram_parameter(
        "x1col", [27, A1_ROWS, W384], F32R, isOutput=False)
    params["w1"] = nc.declare_dram_parameter("w1", [27, 64], F32R, isOutput=False)
    for i, (ci, co) in enumerate(_CHANS):
        if i > 0:
            params[f"w{i + 1}"] = nc.declare_dram_parameter(
                f"w{i + 1}", [ci, 9, co], F32R, isOutput=False)
        params[f"b{i + 1}"] = nc.declare_dram_parameter(
            f"b{i + 1}", [min(co, 128), (co + 127) // 128], F32, isOutput=False)
    params["m_u8"] = nc.declare_dram_parameter(
        "m_u8", [128, ROWS192["A2"], W192], U8, isOutput=False)
    params["rm384"] = nc.declare_dram_parameter(
        "rm384", [128, CANVAS_ROWS], F32, isOutput=False)
    params["rm192"] = nc.declare_dram_parameter(
        "rm192", [128, ROWS192["A2"]], F32, isOutput=False)

    bufs = {"X1": params["x1col"]}
    bufs["A1"] = nc.dram_tensor("A1", [64, A1_ROWS, A1_WP], F32R)
    for name, cc in (("A2", 64), ("A3", 128), ("A4", 128), ("A5", 256),
                     ("A6", 256), ("A7", 256), ("A8", 512), ("A9", 512)):
        bufs[name] = nc.dram_tensor(name, [cc, ROWS192[name], WP], F32R)
    bufs["OUT"] = nc.declare_dram_parameter(
        "out", [512, ROWS192["OUT"], W192], F32, isOutput=True)

    with tile.TileContext(nc) as tc:
        from contextlib import ExitStack
        with ExitStack() as ctx:
            pools = {}
            for name, kw in (
                ("const", dict(bufs=1)),
                ("oacc", dict(bufs=4)),
                ("osel", dict(bufs=4)),
                ("pscr", dict(bufs=2)),
                ("psum", dict(bufs=6, space="PSUM")),
            ):
                pools[name] = ctx.enter_context(tc.tile_pool(name=name, **kw))
            # resident masks
            pools["m_u8"] = pools["const"].tile([128, ROWS192["A2"], W192], U8,
                                                name="m_u8_t", tag="m_u8")
            nc.sync.dma_start(pools["m_u8"][:], params["m_u8"][:])
            pools["rm192"] = pools["const"].tile([128, ROWS192["A2"]], F32,
                                                 name="rm192_t", tag="rm192")
            nc.sync.dma_start(pools["rm192"][:], params["rm192"][:])

            _emit_zero_pads(nc, tc, pools, bufs)
            _emit_l1(nc, tc, pools, bufs, params)
            _emit_l2(nc, tc, pools, bufs, params)
            with tc.tile_pool(name="wres", bufs=1) as p_wres:
                pools["wres"] = p_wres
                for lay in LAYERS[:5]:
                    _emit_conv192(nc, tc, pools, bufs, params, *lay)
            with tc.tile_pool(name="wresC", bufs=1) as p_wres2:
                pools["wres"] = p_wres2
                for lay in LAYERS[5:]:
                    _emit_conv192(nc, tc, pools, bufs, params, *lay)
    nc.compile()
    return nc


# ---------------------------------------------------------------- host side

def _upsample_mask(m48):
    return np.repeat(np.repeat(m48, 4, axis=0), 4, axis=1)


def make_core_inputs(inputs, core):
    b, s = core // 4, core % 4
    r0, R0 = 48 * s, 96 * s
    x = np.asarray(inputs["batch"][b], np.float32)  # [3, 384, 384]

    canvas = np.zeros((3, CANVAS_ROWS, W384 + 2), np.float32)
    lo, hi = R0 - 42, R0 + 138
    clo, chi = max(lo, 0), min(hi, W384)
    canvas[:, clo - lo : chi - lo, 1 : 1 + W384] = x[:, clo:chi, :]

    x1col = np.empty((27, A1_ROWS, W384), np.float32)
    for t, (a, bb) in enumerate(TAPS):
        x1col[3 * t : 3 * t + 3] = canvas[:, 1 + a : 1 + a + A1_ROWS,
                                          1 + bb : 1 + bb + W384]

    m192 = _upsample_mask(np.asarray(inputs["pooling_mask"][b, 0]))  # [192,192]
    mbuf = np.zeros((ROWS192["A2"], W192), np.uint8)
    mlo, mhi = r0 - 20, r0 + 68
    cmlo, cmhi = max(mlo, 0), min(mhi, W192)
    mbuf[cmlo - mlo : cmhi - mlo] = m192[cmlo:cmhi].astype(np.uint8)

    rm384 = ((np.arange(CANVAS_ROWS) + R0 - 42 >= 0)
             & (np.arange(CANVAS_ROWS) + R0 - 42 < W384)).astype(np.float32)
    rm192 = ((np.arange(ROWS192["A2"]) + r0 - 20 >= 0)
             & (np.arange(ROWS192["A2"]) + r0 - 20 < W192)).astype(np.float32)

    im = {
        "x1col": x1col,
        "m_u8": np.broadcast_to(mbuf, (128,) + mbuf.shape).copy(),
        "rm384": np.broadcast_to(rm384, (128, CANVAS_ROWS)).copy(),
        "rm192": np.broadcast_to(rm192, (128, ROWS192["A2"])).copy(),
    }
    w1 = np.asarray(inputs["w1"], np.float32)  # [64, 3, 3, 3]
    w1r = np.empty((27, 64), np.float32)
    for t, (a, bb) in enumerate(TAPS):
        w1r[3 * t : 3 * t + 3] = w1[:, :, a + 1, bb + 1].T
    im["w1"] = w1r
    for i in range(2, 11):
        w = np.asarray(inputs[f"w{i}"], np.float32)  # [O, I, 3, 3]
        im[f"w{i}"] = np.ascontiguousarray(
            w.transpose(1, 2, 3, 0).reshape(w.shape[1], 9, w.shape[0]))
    for i in range(1, 11):
        bv = np.asarray(inputs[f"b{i}"], np.float32)
        im[f"b{i}"] = np.ascontiguousarray(bv.reshape(-1, min(bv.size, 128)).T)
    return im


_NC_CACHE = []


def _get_program():
    if not _NC_CACHE:
        _NC_CACHE.append(build_program())
    return _NC_CACHE[0]


def kernel(**inputs):
    nc = _get_program()
    in_maps = [make_core_inputs(inputs, c) for c in range(8)]
    res = run_bass_kernel_spmd(nc, in_maps, list(range(8)))
    out = np.empty((2, 512, W192, W192), np.float32)
    for c in range(8):
        b, s = c // 4, c % 4
        out[b, :, 48 * s : 48 * s + 48, :] = res.results[c]["out"]
    return out



# revision 2
# speedup vs baseline: 1.4336x; 1.4336x over previous
"""Trainium2 Bass kernel for DenseFeatureExtractionModule (irregular-pooled VGG).

Sharding: 8 cores = 2 images x 4 row-strips of the 192-grid output (48 rows
each). Each core receives its input strip with enough halo rows to compute
all 10 conv layers locally (no inter-core communication). Out-of-image halo
rows are kept at zero through the layer stack by multiplying edge-band rows
with a per-core row-validity mask, which reproduces SAME-conv zero padding.

V2: all activations + weights in bf16 (fp32 PSUM accumulation). Graph-conv
layers (5-10) use a gather-select path: per (cin-tile, tap) one bf16
tensor_copy (4x DVE mode) of the dilation-1 shifted window plus one
copy_predicated overlay of the dilation-s window, then a single set of
matmuls — halving PE work vs dual-dilation and cutting DVE traffic ~3.5x
vs the V1 gather. bf16 weights enable PE fast-weight-load.
"""

import numpy as np
import ml_dtypes

import concourse.bacc as bacc
import concourse.bass as bass
import concourse.mybir as mybir
import concourse.tile as tile
from concourse.bass_utils import run_bass_kernel_spmd

F32 = mybir.dt.float32
BF16 = mybir.dt.bfloat16
U8 = mybir.dt.uint8
RELU = mybir.ActivationFunctionType.Relu
MAX = mybir.AluOpType.max
NPBF16 = ml_dtypes.bfloat16

W192 = 192
W384 = 384
PAD = 4  # pad columns for all 192-grid buffers
WP = W192 + 2 * PAD  # 200
A1_WP = W384 + 2  # 386, pad 1

# buffer row counts (per core strips, incl. halo)
CANVAS_ROWS = 180  # batch strip on 384 grid (96 + 2*42)
A1_ROWS = 178
ROWS192 = {"A2": 88, "A3": 86, "A4": 84, "A5": 80, "A6": 76, "A7": 72,
           "A8": 64, "A9": 56, "OUT": 48}
HALO192 = {"A2": 20, "A3": 19, "A4": 18, "A5": 16, "A6": 14, "A7": 12,
           "A8": 8, "A9": 4, "OUT": 0}

_CHANS = [(3, 64), (64, 64), (64, 128), (128, 128), (128, 256),
          (256, 256), (256, 256), (256, 512), (512, 512), (512, 512)]

# (src, dst, Cin, Cout, dils, pool_s, widx) for layers 3..10
LAYERS = [
    ("A2", "A3", 64, 128, (1,), None, 3),
    ("A3", "A4", 128, 128, (1,), 2, 4),
    ("A4", "A5", 128, 256, (1, 2), None, 5),
    ("A5", "A6", 256, 256, (1, 2), None, 6),
    ("A6", "A7", 256, 256, (1, 2), 4, 7),
    ("A7", "A8", 256, 512, (1, 4), None, 8),
    ("A8", "A9", 512, 512, (1, 4), None, 9),
    ("A9", "OUT", 512, 512, (1, 4), None, 10),
]
RB = 8  # output rows per input-tile block (dense 192-grid layers)
GB = 4  # output rows per block in the gather path

TAPS = [(a, b) for a in (-1, 0, 1) for b in (-1, 0, 1)]


def _emit_zero_pads(nc, tc, pools, bufs):
    """Zero the pad columns of every 192-grid DRAM buffer (and A1)."""
    zt = pools["const"].tile([128, ROWS192["A2"] * PAD], BF16)
    nc.gpsimd.memset(zt[:], 0.0)
    for name, dram in bufs.items():
        if name in ("X1", "OUT"):
            continue
        C, rows, wp = dram.shape
        pad = 1 if name == "A1" else PAD
        w = wp - 2 * pad
        for ci in range((C + 127) // 128):
            p = min(128, C - ci * 128)
            src = zt[:p, : rows * pad].rearrange("p (r c) -> p r c", c=pad)
            nc.sync.dma_start(dram[ci * 128 : ci * 128 + p, :, 0:pad], src)
            nc.sync.dma_start(dram[ci * 128 : ci * 128 + p, :, wp - pad : wp], src)


def _emit_l1(nc, tc, pools, bufs, params):
    """L1: 1x1 conv over im2col'd input (K=27, M=64), 384 grid."""
    x1, a1 = bufs["X1"], bufs["A1"]
    wsb = pools["const"].tile([27, 64], BF16)
    nc.sync.dma_start(wsb[:], params["w1"][:])
    bsb = pools["const"].tile([64, 1], F32)
    nc.sync.dma_start(bsb[:], params["b1"][:])
    rm = pools["const"].tile([128, CANVAS_ROWS], BF16)
    nc.sync.dma_start(rm[:], params["rm384"][:])

    RB1 = 16
    with tc.tile_pool(name="in1", bufs=2) as p_in1, \
         tc.tile_pool(name="out1", bufs=4) as p_out1:
     for i0 in range(0, A1_ROWS, RB1):
        nr = min(RB1, A1_ROWS - i0)
        xt = p_in1.tile([27, RB1, W384], BF16, tag="x1t")
        nc.sync.dma_start(xt[:, :nr, :], x1[:, i0 : i0 + nr, :])
        for j in range(nr):
            i = i0 + j
            ps = pools["psum"].tile([64, W384], F32, tag="ps")
            nc.tensor.matmul(ps[:], wsb[:], xt[:, j, :], start=True, stop=True)
            ot = p_out1.tile([64, W384], BF16, tag="o1")
            nc.scalar.activation(ot[:], ps[:], RELU, bias=bsb[:])
            if i < 41 or i >= A1_ROWS - 41:
                nc.vector.tensor_mul(
                    ot[:], ot[:],
                    rm[0:64, i + 1 : i + 2].broadcast_to([64, W384]))
            nc.sync.dma_start(a1[:, i, 1 : 1 + W384], ot[:])


def _emit_l2(nc, tc, pools, bufs, params):
    """L2 conv (64->64, 384 grid) fused with 2x2 maxpool -> A2 (192 grid)."""
    a1, a2 = bufs["A1"], bufs["A2"]
    wsb = pools["const"].tile([64, 9, 64], BF16)
    nc.sync.dma_start(wsb[:], params["w2"][:])
    bsb = pools["const"].tile([64, 1], F32)
    nc.sync.dma_start(bsb[:], params["b2"][:])
    rm = pools["rm192"]

    QB = 8  # A2 rows per block
    with tc.tile_pool(name="in2", bufs=2) as p_in2, \
         tc.tile_pool(name="out2", bufs=4) as p_out2:
     for q0 in range(0, ROWS192["A2"], QB):
        nq = min(QB, ROWS192["A2"] - q0)
        xt = p_in2.tile([64, 2 * QB + 2, A1_WP], BF16, tag="x2t")
        nc.sync.dma_start(xt[:, : 2 * nq + 2, :],
                          a1[:, 2 * q0 : 2 * q0 + 2 * nq + 2, :])
        for q in range(q0, q0 + nq):
            o2 = p_out2.tile([64, 2, W384], BF16, tag="o2")
            for r in range(2):
                ps = pools["psum"].tile([64, W384], F32, tag="ps")
                for ti, (a, b) in enumerate(TAPS):
                    rhs = xt[:, 2 * (q - q0) + r + 1 + a, 1 + b : 1 + b + W384]
                    nc.tensor.matmul(ps[:], wsb[:, ti, :], rhs,
                                     start=(ti == 0), stop=(ti == 8))
                nc.scalar.activation(o2[:, r, :], ps[:], RELU, bias=bsb[:])
            o2v = o2[:].rearrange("p r (c t) -> p r c t", t=2)
            cm = p_out2.tile([64, 2, W192], BF16, tag="cm2")
            nc.vector.tensor_tensor(cm[:, 0, :], o2v[:, 0, :, 0], o2v[:, 0, :, 1], MAX)
            nc.vector.tensor_tensor(cm[:, 1, :], o2v[:, 1, :, 0], o2v[:, 1, :, 1], MAX)
            mp = p_out2.tile([64, W192], BF16, tag="mp2")
            nc.vector.tensor_tensor(mp[:], cm[:, 0, :], cm[:, 1, :], MAX)
            if q < 20 or q >= ROWS192["A2"] - 20:
                nc.vector.tensor_mul(
                    mp[:], mp[:], rm[0:64, q : q + 1].broadcast_to([64, W192]))
            nc.sync.dma_start(a2[:, q, PAD : PAD + W192], mp[:])


def _emit_dense192(nc, tc, pools, bufs, params, src, dst, cin, cout, dils,
                   pool_s, widx):
    """Dense 192-grid conv layer (single dilation), optional fused pool."""
    sdram, ddram = bufs[src], bufs[dst]
    rows_out = ROWS192[dst]
    h_out = HALO192[dst]
    nci = (cin + 127) // 128
    nco = (cout + 127) // 128
    off = 20 - h_out
    rm = pools["rm192"]
    msb = pools["m_u8"]

    wts = []
    for ci in range(nci):
        p = min(128, cin - ci * 128)
        wt = pools["wres"].tile([p, 9, cout], BF16, name=f"w{widx}_{ci}",
                                tag=f"wres_{ci}")
        nc.sync.dma_start(wt[:], params[f"w{widx}"][ci * 128 : ci * 128 + p])
        wts.append(wt)
    bsb = pools["const"].tile([min(cout, 128), nco], F32, name=f"bsb{widx}")
    nc.sync.dma_start(bsb[:], params[f"b{widx}"][:])

    grp = pool_s if pool_s else 2  # rows per output tile group
    with tc.tile_pool(name=f"xin{widx}", bufs=2) as p_xin:
     for j0 in range(0, rows_out, RB):
        rb = min(RB, rows_out - j0)
        xts = []
        for ci in range(nci):
            p = min(128, cin - ci * 128)
            xt = p_xin.tile([p, RB + 2, WP], BF16, tag=f"xin{ci}")
            nc.sync.dma_start(xt[:, : rb + 2, :],
                              sdram[ci * 128 : ci * 128 + p, j0 : j0 + rb + 2, :])
            xts.append(xt)
        for co in range(nco):
            pco = min(128, cout - co * 128)
            for g0 in range(0, rb, grp):
                tg = pools["oacc"].tile([pco, grp, W192], BF16, tag="oacc")
                for rp in range(grp // 2):
                    j = j0 + g0 + rp * 2
                    ps = pools["psum"].tile([pco, 2 * W192], F32, tag="ps")
                    for ci in range(nci):
                        for ti, (a, b) in enumerate(TAPS):
                            rhs = xts[ci][:, g0 + rp * 2 + 1 + a :
                                          g0 + rp * 2 + 1 + a + 2,
                                          PAD + b : PAD + b + W192]
                            nc.tensor.matmul(
                                ps[:],
                                wts[ci][:, ti, co * 128 : co * 128 + pco],
                                rhs,
                                start=(ci == 0 and ti == 0),
                                stop=(ci == nci - 1 and ti == 8))
                    t1 = tg[:, rp * 2 : rp * 2 + 2, :]
                    psv = ps[:].rearrange("p (r w) -> p r w", w=W192)
                    nc.scalar.activation(t1, psv, RELU, bias=bsb[:pco, co : co + 1])
                    if j < h_out or j + 2 > rows_out - h_out:
                        nc.vector.tensor_mul(
                            t1, t1,
                            rm[:pco, off + j : off + j + 2].unsqueeze(-1)
                            .broadcast_to([pco, 2, W192]))
                j = j0 + g0
                if pool_s == 2:
                    tv = tg[:].rearrange("p r (c t) -> p r c t", t=2)
                    cm = pools["pscr"].tile([pco, 2, W192 // 2], BF16, tag="pcm")
                    nc.vector.tensor_tensor(cm[:], tv[:, :, :, 0], tv[:, :, :, 1], MAX)
                    bm = pools["pscr"].tile([pco, W192 // 2], BF16, tag="pbm")
                    nc.vector.tensor_tensor(bm[:], cm[:, 0, :], cm[:, 1, :], MAX)
                    rep = pools["pscr"].tile([pco, 2, W192], BF16, tag="prep")
                    nc.vector.tensor_copy(
                        rep[:], bm[:].unsqueeze(1).unsqueeze(-1)
                        .broadcast_to([pco, 2, W192 // 2, 2]))
                    nc.vector.copy_predicated(
                        tg[:], msb[:pco, off + j : off + j + 2, :], rep[:])
                nc.sync.dma_start(
                    ddram[co * 128 : co * 128 + pco, j : j + grp,
                          PAD : PAD + W192], tg[:])


def _emit_gather192(nc, tc, pools, bufs, params, src, dst, cin, cout, dils,
                    pool_s, widx):
    """Graph-conv layer via gather-select: build the per-tap selected input
    window (d1 copy + masked d-s overlay) once per (cin-tile, tap), then a
    single matmul set. Optional fused s=4 irregular pool on the output."""
    sdram, ddram = bufs[src], bufs[dst]
    rows_out = ROWS192[dst]
    h_out = HALO192[dst]
    s = dils[1]
    dm = s
    nci = (cin + 127) // 128
    nco = (cout + 127) // 128
    off = 20 - h_out
    act_dt = F32 if dst == "OUT" else BF16
    rm = pools["rm192"]
    msb = pools["m_u8"]
    dst_c0 = 0 if dst == "OUT" else PAD

    wts = []
    for ci in range(nci):
        wt = pools["wres"].tile([128, 9, cout], BF16, name=f"w{widx}_{ci}",
                                tag=f"wres_{ci}")
        nc.sync.dma_start(wt[:], params[f"w{widx}"][ci * 128 : ci * 128 + 128])
        wts.append(wt)
    bsb = pools["const"].tile([min(cout, 128), nco], F32, name=f"bsb{widx}")
    nc.sync.dma_start(bsb[:], params[f"b{widx}"][:])

    with tc.tile_pool(name=f"xin{widx}", bufs=2) as p_xin, \
         tc.tile_pool(name=f"gp{widx}", bufs=6) as p_g:
        for j0 in range(0, rows_out, GB):
            xts = []
            for ci in range(nci):
                xt = p_xin.tile([128, GB + 2 * dm, WP], BF16, tag=f"xin{ci}")
                nc.sync.dma_start(
                    xt[:], sdram[ci * 128 : ci * 128 + 128,
                                 j0 : j0 + GB + 2 * dm, :])
                xts.append(xt)
            pss = [[pools["psum"].tile([128, 2 * W192], F32, tag="ps",
                                       name=f"ps{widx}_{j0}_{co}_{h}")
                    for h in range(GB // 2)] for co in range(nco)]
            for ci in range(nci):
                for ti, (a, b) in enumerate(TAPS):
                    g = p_g.tile([128, GB, W192], BF16, tag="g3")
                    nc.vector.tensor_copy(
                        g[:], xts[ci][:, dm + a : dm + a + GB,
                                      PAD + b : PAD + b + W192])
                    nc.vector.copy_predicated(
                        g[:], msb[:, off + j0 : off + j0 + GB, :],
                        xts[ci][:, dm + a * s : dm + a * s + GB,
                                PAD + b * s : PAD + b * s + W192])
                    gv = g[:].rearrange("p r w -> p (r w)")
                    for co in range(nco):
                        pco = min(128, cout - co * 128)
                        for h in range(GB // 2):
                            nc.tensor.matmul(
                                pss[co][h][:pco, :],
                                wts[ci][:, ti, co * 128 : co * 128 + pco],
                                gv[:, h * 2 * W192 : (h + 1) * 2 * W192],
                                start=(ci == 0 and ti == 0),
                                stop=(ci == nci - 1 and ti == 8))
            for co in range(nco):
                pco = min(128, cout - co * 128)
                tg = pools["oacc"].tile([pco, GB, W192], act_dt, tag="oacc")
                for h in range(GB // 2):
                    psv = pss[co][h][:pco, :].rearrange("p (r w) -> p r w", w=W192)
                    nc.scalar.activation(tg[:, h * 2 : h * 2 + 2, :], psv, RELU,
                                         bias=bsb[:pco, co : co + 1])
                if j0 < h_out or j0 + GB > rows_out - h_out:
                    nc.vector.tensor_mul(
                        tg[:], tg[:],
                        rm[:pco, off + j0 : off + j0 + GB].unsqueeze(-1)
                        .broadcast_to([pco, GB, W192]))
                if pool_s == 4:
                    tv = tg[:].rearrange("p r (c t) -> p r c t", t=4)
                    c1 = pools["pscr"].tile([pco, 4, W192 // 4], BF16, tag="pc1")
                    c2 = pools["pscr"].tile([pco, 4, W192 // 4], BF16, tag="pc2")
                    nc.vector.tensor_tensor(c1[:], tv[:, :, :, 0], tv[:, :, :, 1], MAX)
                    nc.vector.tensor_tensor(c2[:], tv[:, :, :, 2], tv[:, :, :, 3], MAX)
                    nc.vector.tensor_tensor(c1[:], c1[:], c2[:], MAX)
                    r1 = pools["pscr"].tile([pco, W192 // 4], BF16, tag="pr1")
                    r2 = pools["pscr"].tile([pco, W192 // 4], BF16, tag="pr2")
                    nc.vector.tensor_tensor(r1[:], c1[:, 0, :], c1[:, 1, :], MAX)
                    nc.vector.tensor_tensor(r2[:], c1[:, 2, :], c1[:, 3, :], MAX)
                    nc.vector.tensor_tensor(r1[:], r1[:], r2[:], MAX)
                    rep = pools["pscr"].tile([pco, 4, W192], BF16, tag="prep4")
                    nc.vector.tensor_copy(
                        rep[:], r1[:].unsqueeze(1).unsqueeze(-1)
                        .broadcast_to([pco, 4, W192 // 4, 4]))
                    nc.vector.copy_predicated(
                        tg[:], msb[:pco, off + j0 : off + j0 + 4, :], rep[:])
                nc.sync.dma_start(
                    ddram[co * 128 : co * 128 + pco, j0 : j0 + GB,
                          dst_c0 : dst_c0 + W192], tg[:])


def build_program():
    nc = bacc.Bacc()
    params = {}
    params["x1col"] = nc.declare_dram_parameter(
        "x1col", [27, A1_ROWS, W384], BF16, isOutput=False)
    params["w1"] = nc.declare_dram_parameter("w1", [27, 64], BF16, isOutput=False)
    for i, (ci, co) in enumerate(_CHANS):
        if i > 0:
            params[f"w{i + 1}"] = nc.declare_dram_parameter(
                f"w{i + 1}", [ci, 9, co], BF16, isOutput=False)
        params[f"b{i + 1}"] = nc.declare_dram_parameter(
            f"b{i + 1}", [min(co, 128), (co + 127) // 128], F32, isOutput=False)
    params["m_u8"] = nc.declare_dram_parameter(
        "m_u8", [128, ROWS192["A2"], W192], U8, isOutput=False)
    params["rm384"] = nc.declare_dram_parameter(
        "rm384", [128, CANVAS_ROWS], BF16, isOutput=False)
    params["rm192"] = nc.declare_dram_parameter(
        "rm192", [128, ROWS192["A2"]], BF16, isOutput=False)

    bufs = {"X1": params["x1col"]}
    bufs["A1"] = nc.dram_tensor("A1", [64, A1_ROWS, A1_WP], BF16)
    for name, cc in (("A2", 64), ("A3", 128), ("A4", 128), ("A5", 256),
                     ("A6", 256), ("A7", 256), ("A8", 512), ("A9", 512)):
        bufs[name] = nc.dram_tensor(name, [cc, ROWS192[name], WP], BF16)
    bufs["OUT"] = nc.declare_dram_parameter(
        "out", [512, ROWS192["OUT"], W192], F32, isOutput=True)

    with tile.TileContext(nc) as tc:
        from contextlib import ExitStack
        with ExitStack() as ctx:
            pools = {}
            for name, kw in (
                ("const", dict(bufs=1)),
                ("oacc", dict(bufs=4)),
                ("pscr", dict(bufs=2)),
                ("psum", dict(bufs=8, space="PSUM")),
            ):
                pools[name] = ctx.enter_context(tc.tile_pool(name=name, **kw))
            # resident masks
            pools["m_u8"] = pools["const"].tile([128, ROWS192["A2"], W192], U8,
                                                name="m_u8_t", tag="m_u8")
            nc.sync.dma_start(pools["m_u8"][:], params["m_u8"][:])
            pools["rm192"] = pools["const"].tile([128, ROWS192["A2"]], BF16,
                                                 name="rm192_t", tag="rm192")
            nc.sync.dma_start(pools["rm192"][:], params["rm192"][:])

            _emit_zero_pads(nc, tc, pools, bufs)
            _emit_l1(nc, tc, pools, bufs, params)
            _emit_l2(nc, tc, pools, bufs, params)
            with tc.tile_pool(name="wres", bufs=1) as p_wres:
                pools["wres"] = p_wres
                for lay in LAYERS:
                    if len(lay[4]) == 1:
                        _emit_dense192(nc, tc, pools, bufs, params, *lay)
                    else:
                        _emit_gather192(nc, tc, pools, bufs, params, *lay)
    nc.compile()
    return nc


# ---------------------------------------------------------------- host side

def _upsample_mask(m48):
    return np.repeat(np.repeat(m48, 4, axis=0), 4, axis=1)


def make_core_inputs(inputs, core):
    b, s = core // 4, core % 4
    r0, R0 = 48 * s, 96 * s
    x = np.asarray(inputs["batch"][b], np.float32)  # [3, 384, 384]

    canvas = np.zeros((3, CANVAS_ROWS, W384 + 2), np.float32)
    lo, hi = R0 - 42, R0 + 138
    clo, chi = max(lo, 0), min(hi, W384)
    canvas[:, clo - lo : chi - lo, 1 : 1 + W384] = x[:, clo:chi, :]

    x1col = np.empty((27, A1_ROWS, W384), np.float32)
    for t, (a, bb) in enumerate(TAPS):
        x1col[3 * t : 3 * t + 3] = canvas[:, 1 + a : 1 + a + A1_ROWS,
                                          1 + bb : 1 + bb + W384]

    m192 = _upsample_mask(np.asarray(inputs["pooling_mask"][b, 0]))  # [192,192]
    mbuf = np.zeros((ROWS192["A2"], W192), np.uint8)
    mlo, mhi = r0 - 20, r0 + 68
    cmlo, cmhi = max(mlo, 0), min(mhi, W192)
    mbuf[cmlo - mlo : cmhi - mlo] = m192[cmlo:cmhi].astype(np.uint8)

    rm384 = ((np.arange(CANVAS_ROWS) + R0 - 42 >= 0)
             & (np.arange(CANVAS_ROWS) + R0 - 42 < W384)).astype(np.float32)
    rm192 = ((np.arange(ROWS192["A2"]) + r0 - 20 >= 0)
             & (np.arange(ROWS192["A2"]) + r0 - 20 < W192)).astype(np.float32)

    im = {
        "x1col": x1col.astype(NPBF16),
        "m_u8": np.broadcast_to(mbuf, (128,) + mbuf.shape).copy(),
        "rm384": np.broadcast_to(rm384, (128, CANVAS_ROWS)).astype(NPBF16),
        "rm192": np.broadcast_to(rm192, (128, ROWS192["A2"])).astype(NPBF16),
    }
    w1 = np.asarray(inputs["w1"], np.float32)  # [64, 3, 3, 3]
    w1r = np.empty((27, 64), np.float32)
    for t, (a, bb) in enumerate(TAPS):
        w1r[3 * t : 3 * t + 3] = w1[:, :, a + 1, bb + 1].T
    im["w1"] = w1r.astype(NPBF16)
    for i in range(2, 11):
        w = np.asarray(inputs[f"w{i}"], np.float32)  # [O, I, 3, 3]
        im[f"w{i}"] = np.ascontiguousarray(
            w.transpose(1, 2, 3, 0).reshape(w.shape[1], 9, w.shape[0])
        ).astype(NPBF16)
    for i in range(1, 11):
        bv = np.asarray(inputs[f"b{i}"], np.float32)
        im[f"b{i}"] = np.ascontiguousarray(bv.reshape(-1, min(bv.size, 128)).T)
    return im


_NC_CACHE = []


def _get_program():
    if not _NC_CACHE:
        _NC_CACHE.append(build_program())
    return _NC_CACHE[0]


def kernel(**inputs):
    nc = _get_program()
    in_maps = [make_core_inputs(inputs, c) for c in range(8)]
    res = run_bass_kernel_spmd(nc, in_maps, list(range(8)))
    out = np.empty((2, 512, W192, W192), np.float32)
    for c in range(8):
        b, s = c // 4, c % 4
        out[b, :, 48 * s : 48 * s + 48, :] = res.results[c]["out"]
    return out


# revision 11
# speedup vs baseline: 1.5037x; 1.0489x over previous
"""Trainium2 Bass kernel for DenseFeatureExtractionModule (irregular-pooled VGG).

Sharding: 8 cores = 2 images x 4 row-strips of the 192-grid output (48 rows
each). Each core receives its input strip with enough halo rows to compute
all 10 conv layers locally (no inter-core communication). Out-of-image halo
rows are kept at zero through the layer stack by multiplying edge-band rows
with a per-core row-validity mask, which reproduces SAME-conv zero padding.

V2: all activations + weights in bf16 (fp32 PSUM accumulation). Graph-conv
layers (5-10) use a gather-select path: per (cin-tile, tap) one bf16
tensor_copy (4x DVE mode) of the dilation-1 shifted window plus one
copy_predicated overlay of the dilation-s window, then a single set of
matmuls — halving PE work vs dual-dilation and cutting DVE traffic ~3.5x
vs the V1 gather. bf16 weights enable PE fast-weight-load.
"""

import numpy as np
import ml_dtypes

import concourse.bacc as bacc
import concourse.bass as bass
import concourse.mybir as mybir
import concourse.tile as tile
from concourse.bass_utils import run_bass_kernel_spmd

F32 = mybir.dt.float32
BF16 = mybir.dt.bfloat16
U8 = mybir.dt.uint8
RELU = mybir.ActivationFunctionType.Relu
MAX = mybir.AluOpType.max
NPBF16 = ml_dtypes.bfloat16

W192 = 192
W384 = 384
PAD = 4  # pad columns for all 192-grid buffers
WP = W192 + 2 * PAD  # 200
A1_WP = W384 + 2  # 386, pad 1

# buffer row counts (per core strips, incl. halo)
CANVAS_ROWS = 180  # batch strip on 384 grid (96 + 2*42)
A1_ROWS = 178
ROWS192 = {"A2": 88, "A3": 86, "A4": 84, "A5": 80, "A6": 76, "A7": 72,
           "A8": 64, "A9": 56, "OUT": 48}
HALO192 = {"A2": 20, "A3": 19, "A4": 18, "A5": 16, "A6": 14, "A7": 12,
           "A8": 8, "A9": 4, "OUT": 0}

_CHANS = [(3, 64), (64, 64), (64, 128), (128, 128), (128, 256),
          (256, 256), (256, 256), (256, 512), (512, 512), (512, 512)]

# (src, dst, Cin, Cout, dils, pool_s, widx) for layers 3..10
LAYERS = [
    ("A2", "A3", 64, 128, (1,), None, 3),
    ("A3", "A4", 128, 128, (1,), 2, 4),
    ("A4", "A5", 128, 256, (1, 2), None, 5),
    ("A5", "A6", 256, 256, (1, 2), None, 6),
    ("A6", "A7", 256, 256, (1, 2), 4, 7),
    ("A7", "A8", 256, 512, (1, 4), None, 8),
    ("A8", "A9", 512, 512, (1, 4), None, 9),
    ("A9", "OUT", 512, 512, (1, 4), None, 10),
]
RB = 8  # output rows per input-tile block (dense 192-grid layers)
GB = 4  # output rows per block in the gather path

TAPS = [(a, b) for a in (-1, 0, 1) for b in (-1, 0, 1)]


def _emit_zero_pads(nc, tc, pools, bufs):
    """Zero the pad columns of every 192-grid DRAM buffer (and A1)."""
    zt = pools["const"].tile([128, ROWS192["A2"] * PAD], BF16)
    nc.gpsimd.memset(zt[:], 0.0)
    for name, dram in bufs.items():
        if name in ("X1", "OUT"):
            continue
        C, rows, wp = dram.shape
        pad = 1 if name == "A1" else PAD
        w = wp - 2 * pad
        for ci in range((C + 127) // 128):
            p = min(128, C - ci * 128)
            src = zt[:p, : rows * pad].rearrange("p (r c) -> p r c", c=pad)
            nc.sync.dma_start(dram[ci * 128 : ci * 128 + p, :, 0:pad], src)
            nc.sync.dma_start(dram[ci * 128 : ci * 128 + p, :, wp - pad : wp], src)


def _emit_l1(nc, tc, pools, bufs, params):
    """L1: 1x1 conv over im2col'd input (K=27, M=64), 384 grid."""
    x1, a1 = bufs["X1"], bufs["A1"]
    wsb = pools["const"].tile([27, 64], BF16)
    nc.sync.dma_start(wsb[:], params["w1"][:])
    bsb = pools["const"].tile([64, 1], F32)
    nc.sync.dma_start(bsb[:], params["b1"][:])
    rm = pools["const"].tile([128, CANVAS_ROWS], BF16)
    nc.sync.dma_start(rm[:], params["rm384"][:])

    RB1 = 16
    with tc.tile_pool(name="in1", bufs=2) as p_in1, \
         tc.tile_pool(name="out1", bufs=4) as p_out1:
     for i0 in range(0, A1_ROWS, RB1):
        nr = min(RB1, A1_ROWS - i0)
        xt = p_in1.tile([27, RB1, W384], BF16, tag="x1t")
        nc.sync.dma_start(xt[:, :nr, :], x1[:, i0 : i0 + nr, :])
        for j in range(nr):
            i = i0 + j
            ps = pools["psum"].tile([64, W384], F32, tag="ps")
            nc.tensor.matmul(ps[:], wsb[:], xt[:, j, :], start=True, stop=True)
            ot = p_out1.tile([64, W384], BF16, tag="o1")
            nc.scalar.activation(ot[:], ps[:], RELU, bias=bsb[:])
            if i < 41 or i >= A1_ROWS - 41:
                nc.gpsimd.tensor_mul(
                    ot[:], ot[:],
                    rm[0:64, i + 1 : i + 2].broadcast_to([64, W384]))
            nc.sync.dma_start(a1[:, i, 1 : 1 + W384], ot[:])


def _emit_l2(nc, tc, pools, bufs, params):
    """L2 conv (64->64, 384 grid) fused with 2x2 maxpool -> A2 (192 grid)."""
    a1, a2 = bufs["A1"], bufs["A2"]
    wsb = pools["const"].tile([64, 9, 64], BF16)
    nc.sync.dma_start(wsb[:], params["w2"][:])
    bsb = pools["const"].tile([64, 1], F32)
    nc.sync.dma_start(bsb[:], params["b2"][:])
    rm = pools["rm192"]

    QB = 8  # A2 rows per block
    with tc.tile_pool(name="in2", bufs=2) as p_in2, \
         tc.tile_pool(name="out2", bufs=4) as p_out2:
     for q0 in range(0, ROWS192["A2"], QB):
        nq = min(QB, ROWS192["A2"] - q0)
        xt = p_in2.tile([64, 2 * QB + 2, A1_WP], BF16, tag="x2t")
        nc.sync.dma_start(xt[:, : 2 * nq + 2, :],
                          a1[:, 2 * q0 : 2 * q0 + 2 * nq + 2, :])
        for q in range(q0, q0 + nq):
            o2 = p_out2.tile([64, 2, W384], BF16, tag="o2")
            for r in range(2):
                ps = pools["psum"].tile([64, W384], F32, tag="ps")
                for ti, (a, b) in enumerate(TAPS):
                    rhs = xt[:, 2 * (q - q0) + r + 1 + a, 1 + b : 1 + b + W384]
                    nc.tensor.matmul(ps[:], wsb[:, ti, :], rhs,
                                     start=(ti == 0), stop=(ti == 8))
                nc.scalar.activation(o2[:, r, :], ps[:], RELU, bias=bsb[:])
            o2v = o2[:].rearrange("p r (c t) -> p r c t", t=2)
            cm = p_out2.tile([64, 2, W192], BF16, tag="cm2")
            nc.vector.tensor_tensor(cm[:, 0, :], o2v[:, 0, :, 0], o2v[:, 0, :, 1], MAX)
            nc.vector.tensor_tensor(cm[:, 1, :], o2v[:, 1, :, 0], o2v[:, 1, :, 1], MAX)
            mp = p_out2.tile([64, W192], BF16, tag="mp2")
            nc.vector.tensor_tensor(mp[:], cm[:, 0, :], cm[:, 1, :], MAX)
            if q < 20 or q >= ROWS192["A2"] - 20:
                nc.gpsimd.tensor_mul(
                    mp[:], mp[:], rm[0:64, q : q + 1].broadcast_to([64, W192]))
            nc.sync.dma_start(a2[:, q, PAD : PAD + W192], mp[:])


def _emit_dense192(nc, tc, pools, bufs, params, src, dst, cin, cout, dils,
                   pool_s, widx):
    """Dense 192-grid conv layer (single dilation), optional fused pool."""
    sdram, ddram = bufs[src], bufs[dst]
    rows_out = ROWS192[dst]
    h_out = HALO192[dst]
    nci = (cin + 127) // 128
    nco = (cout + 127) // 128
    off = 20 - h_out
    rm = pools["rm192"]
    msb = pools["m_u8"]

    wts = []
    for ci in range(nci):
        p = min(128, cin - ci * 128)
        wt = pools["wres"].tile([p, 9, cout], BF16, name=f"w{widx}_{ci}",
                                tag=f"wres_{ci}")
        nc.sync.dma_start(wt[:], params[f"w{widx}"][ci * 128 : ci * 128 + p])
        wts.append(wt)
    bsb = pools["const"].tile([min(cout, 128), nco], F32, name=f"bsb{widx}")
    nc.sync.dma_start(bsb[:], params[f"b{widx}"][:])

    grp = pool_s if pool_s else 2  # rows per output tile group
    with tc.tile_pool(name=f"xin{widx}", bufs=2) as p_xin:
     for j0 in range(0, rows_out, RB):
        rb = min(RB, rows_out - j0)
        xts = []
        for ci in range(nci):
            p = min(128, cin - ci * 128)
            xt = p_xin.tile([p, RB + 2, WP], BF16, tag=f"xin{ci}")
            nc.sync.dma_start(xt[:, : rb + 2, :],
                              sdram[ci * 128 : ci * 128 + p, j0 : j0 + rb + 2, :])
            xts.append(xt)
        for co in range(nco):
            pco = min(128, cout - co * 128)
            for g0 in range(0, rb, grp):
                tg = pools["oacc"].tile([pco, grp, W192], BF16, tag="oacc")
                for rp in range(grp // 2):
                    j = j0 + g0 + rp * 2
                    ps = pools["psum"].tile([pco, 2 * W192], F32, tag="ps")
                    for ci in range(nci):
                        for ti, (a, b) in enumerate(TAPS):
                            rhs = xts[ci][:, g0 + rp * 2 + 1 + a :
                                          g0 + rp * 2 + 1 + a + 2,
                                          PAD + b : PAD + b + W192]
                            nc.tensor.matmul(
                                ps[:],
                                wts[ci][:, ti, co * 128 : co * 128 + pco],
                                rhs,
                                start=(ci == 0 and ti == 0),
                                stop=(ci == nci - 1 and ti == 8))
                    t1 = tg[:, rp * 2 : rp * 2 + 2, :]
                    psv = ps[:].rearrange("p (r w) -> p r w", w=W192)
                    nc.scalar.activation(t1, psv, RELU, bias=bsb[:pco, co : co + 1])
                    if j < h_out or j + 2 > rows_out - h_out:
                        nc.gpsimd.tensor_mul(
                            t1, t1,
                            rm[:pco, off + j : off + j + 2].unsqueeze(-1)
                            .broadcast_to([pco, 2, W192]))
                j = j0 + g0
                if pool_s == 2:
                    tv = tg[:].rearrange("p r (c t) -> p r c t", t=2)
                    cm = pools["pscr"].tile([pco, 2, W192 // 2], BF16, tag="pcm")
                    nc.vector.tensor_tensor(cm[:], tv[:, :, :, 0], tv[:, :, :, 1], MAX)
                    bm = pools["pscr"].tile([pco, W192 // 2], BF16, tag="pbm")
                    nc.vector.tensor_tensor(bm[:], cm[:, 0, :], cm[:, 1, :], MAX)
                    rep = pools["pscr"].tile([pco, 2, W192], BF16, tag="prep")
                    nc.vector.tensor_copy(
                        rep[:], bm[:].unsqueeze(1).unsqueeze(-1)
                        .broadcast_to([pco, 2, W192 // 2, 2]))
                    nc.vector.copy_predicated(
                        tg[:], msb[:pco, off + j : off + j + 2, :], rep[:])
                nc.sync.dma_start(
                    ddram[co * 128 : co * 128 + pco, j : j + grp,
                          PAD : PAD + W192], tg[:])


def _emit_gather192(nc, tc, pools, bufs, params, src, dst, cin, cout, dils,
                    pool_s, widx):
    """Graph-conv layer via gather-select: build the per-tap selected input
    window (d1 copy + masked d-s overlay) once per (cin-tile, tap), then a
    single matmul set. Optional fused s=4 irregular pool on the output."""
    sdram, ddram = bufs[src], bufs[dst]
    rows_out = ROWS192[dst]
    h_out = HALO192[dst]
    s = dils[1]
    dm = s
    nci = (cin + 127) // 128
    nco = (cout + 127) // 128
    off = 20 - h_out
    act_dt = F32 if dst == "OUT" else BF16
    rm = pools["rm192"]
    msb = pools["m_u8"]
    dst_c0 = 0 if dst == "OUT" else PAD

    wts = []
    for ci in range(nci):
        wt = pools["wres"].tile([128, 9, cout], BF16, name=f"w{widx}_{ci}",
                                tag=f"wres_{ci}")
        nc.sync.dma_start(wt[:], params[f"w{widx}"][ci * 128 : ci * 128 + 128])
        wts.append(wt)
    bsb = pools["const"].tile([min(cout, 128), nco], F32, name=f"bsb{widx}")
    nc.sync.dma_start(bsb[:], params[f"b{widx}"][:])

    with tc.tile_pool(name=f"xin{widx}", bufs=2) as p_xin, \
         tc.tile_pool(name=f"gp{widx}", bufs=8) as p_g:
        for j0 in range(0, rows_out, GB):
            xts = []
            for ci in range(nci):
                xt = p_xin.tile([128, GB + 2 * dm, WP], BF16, tag=f"xin{ci}")
                nc.sync.dma_start(
                    xt[:], sdram[ci * 128 : ci * 128 + 128,
                                 j0 : j0 + GB + 2 * dm, :])
                xts.append(xt)
            pss = [[pools["psum"].tile([128, 2 * W192], F32, tag="ps",
                                       name=f"ps{widx}_{j0}_{co}_{h}")
                    for h in range(GB // 2)] for co in range(nco)]
            for ci in range(nci):
                for ti, (a, b) in enumerate(TAPS):
                    if a == 0 and b == 0:
                        # center tap: d1 and d-s windows coincide; feed the
                        # input view straight to the PE
                        gr = xts[ci][:, dm : dm + GB, PAD : PAD + W192]
                    else:
                        g = p_g.tile([128, GB, W192], BF16, tag="g3")
                        nc.vector.tensor_copy(
                            g[:], xts[ci][:, dm + a : dm + a + GB,
                                          PAD + b : PAD + b + W192])
                        nc.vector.copy_predicated(
                            g[:], msb[:, off + j0 : off + j0 + GB, :],
                            xts[ci][:, dm + a * s : dm + a * s + GB,
                                    PAD + b * s : PAD + b * s + W192])
                        gr = g[:]
                    for co in range(nco):
                        pco = min(128, cout - co * 128)
                        for h in range(GB // 2):
                            nc.tensor.matmul(
                                pss[co][h][:pco, :],
                                wts[ci][:, ti, co * 128 : co * 128 + pco],
                                gr[:, h * 2 : h * 2 + 2, :],
                                start=(ci == 0 and ti == 0),
                                stop=(ci == nci - 1 and ti == 8))
            for co in range(nco):
                pco = min(128, cout - co * 128)
                tg = pools["oacc"].tile([pco, GB, W192], act_dt, tag="oacc")
                for h in range(GB // 2):
                    psv = pss[co][h][:pco, :].rearrange("p (r w) -> p r w", w=W192)
                    nc.scalar.activation(tg[:, h * 2 : h * 2 + 2, :], psv, RELU,
                                         bias=bsb[:pco, co : co + 1])
                if j0 < h_out or j0 + GB > rows_out - h_out:
                    nc.gpsimd.tensor_mul(
                        tg[:], tg[:],
                        rm[:pco, off + j0 : off + j0 + GB].unsqueeze(-1)
                        .broadcast_to([pco, GB, W192]))
                if pool_s == 4:
                    tv = tg[:].rearrange("p r (c t) -> p r c t", t=4)
                    c1 = pools["pscr"].tile([pco, 4, W192 // 4], BF16, tag="pc1")
                    c2 = pools["pscr"].tile([pco, 4, W192 // 4], BF16, tag="pc2")
                    nc.vector.tensor_tensor(c1[:], tv[:, :, :, 0], tv[:, :, :, 1], MAX)
                    nc.vector.tensor_tensor(c2[:], tv[:, :, :, 2], tv[:, :, :, 3], MAX)
                    nc.vector.tensor_tensor(c1[:], c1[:], c2[:], MAX)
                    r1 = pools["pscr"].tile([pco, W192 // 4], BF16, tag="pr1")
                    r2 = pools["pscr"].tile([pco, W192 // 4], BF16, tag="pr2")
                    nc.vector.tensor_tensor(r1[:], c1[:, 0, :], c1[:, 1, :], MAX)
                    nc.vector.tensor_tensor(r2[:], c1[:, 2, :], c1[:, 3, :], MAX)
                    nc.vector.tensor_tensor(r1[:], r1[:], r2[:], MAX)
                    rep = pools["pscr"].tile([pco, 4, W192], BF16, tag="prep4")
                    nc.vector.tensor_copy(
                        rep[:], r1[:].unsqueeze(1).unsqueeze(-1)
                        .broadcast_to([pco, 4, W192 // 4, 4]))
                    nc.vector.copy_predicated(
                        tg[:], msb[:pco, off + j0 : off + j0 + 4, :], rep[:])
                nc.sync.dma_start(
                    ddram[co * 128 : co * 128 + pco, j0 : j0 + GB,
                          dst_c0 : dst_c0 + W192], tg[:])


def build_program():
    nc = bacc.Bacc()
    params = {}
    params["x1col"] = nc.declare_dram_parameter(
        "x1col", [27, A1_ROWS, W384], BF16, isOutput=False)
    params["w1"] = nc.declare_dram_parameter("w1", [27, 64], BF16, isOutput=False)
    for i, (ci, co) in enumerate(_CHANS):
        if i > 0:
            params[f"w{i + 1}"] = nc.declare_dram_parameter(
                f"w{i + 1}", [ci, 9, co], BF16, isOutput=False)
        params[f"b{i + 1}"] = nc.declare_dram_parameter(
            f"b{i + 1}", [min(co, 128), (co + 127) // 128], F32, isOutput=False)
    params["m_u8"] = nc.declare_dram_parameter(
        "m_u8", [128, ROWS192["A2"], W192], U8, isOutput=False)
    params["rm384"] = nc.declare_dram_parameter(
        "rm384", [128, CANVAS_ROWS], BF16, isOutput=False)
    params["rm192"] = nc.declare_dram_parameter(
        "rm192", [128, ROWS192["A2"]], BF16, isOutput=False)

    bufs = {"X1": params["x1col"]}
    bufs["A1"] = nc.dram_tensor("A1", [64, A1_ROWS, A1_WP], BF16)
    for name, cc in (("A2", 64), ("A3", 128), ("A4", 128), ("A5", 256),
                     ("A6", 256), ("A7", 256), ("A8", 512), ("A9", 512)):
        bufs[name] = nc.dram_tensor(name, [cc, ROWS192[name], WP], BF16)
    bufs["OUT"] = nc.declare_dram_parameter(
        "out", [512, ROWS192["OUT"], W192], F32, isOutput=True)

    with tile.TileContext(nc) as tc:
        from contextlib import ExitStack
        with ExitStack() as ctx:
            pools = {}
            for name, kw in (
                ("const", dict(bufs=1)),
                ("oacc", dict(bufs=6)),
                ("pscr", dict(bufs=2)),
                ("psum", dict(bufs=8, space="PSUM")),
            ):
                pools[name] = ctx.enter_context(tc.tile_pool(name=name, **kw))
            # resident masks
            pools["m_u8"] = pools["const"].tile([128, ROWS192["A2"], W192], U8,
                                                name="m_u8_t", tag="m_u8")
            nc.sync.dma_start(pools["m_u8"][:], params["m_u8"][:])
            pools["rm192"] = pools["const"].tile([128, ROWS192["A2"]], BF16,
                                                 name="rm192_t", tag="rm192")
            nc.sync.dma_start(pools["rm192"][:], params["rm192"][:])

            _emit_zero_pads(nc, tc, pools, bufs)
            _emit_l1(nc, tc, pools, bufs, params)
            _emit_l2(nc, tc, pools, bufs, params)
            with tc.tile_pool(name="wres", bufs=2) as p_wres:
                pools["wres"] = p_wres
                for lay in LAYERS:
                    if len(lay[4]) == 1:
                        _emit_dense192(nc, tc, pools, bufs, params, *lay)
                    else:
                        _emit_gather192(nc, tc, pools, bufs, params, *lay)
    nc.compile()
    return nc


# ---------------------------------------------------------------- host side

def _upsample_mask(m48):
    return np.repeat(np.repeat(m48, 4, axis=0), 4, axis=1)


def make_core_inputs(inputs, core):
    b, s = core // 4, core % 4
    r0, R0 = 48 * s, 96 * s
    x = np.asarray(inputs["batch"][b], np.float32)  # [3, 384, 384]

    canvas = np.zeros((3, CANVAS_ROWS, W384 + 2), np.float32)
    lo, hi = R0 - 42, R0 + 138
    clo, chi = max(lo, 0), min(hi, W384)
    canvas[:, clo - lo : chi - lo, 1 : 1 + W384] = x[:, clo:chi, :]

    x1col = np.empty((27, A1_ROWS, W384), np.float32)
    for t, (a, bb) in enumerate(TAPS):
        x1col[3 * t : 3 * t + 3] = canvas[:, 1 + a : 1 + a + A1_ROWS,
                                          1 + bb : 1 + bb + W384]

    m192 = _upsample_mask(np.asarray(inputs["pooling_mask"][b, 0]))  # [192,192]
    mbuf = np.zeros((ROWS192["A2"], W192), np.uint8)
    mlo, mhi = r0 - 20, r0 + 68
    cmlo, cmhi = max(mlo, 0), min(mhi, W192)
    mbuf[cmlo - mlo : cmhi - mlo] = m192[cmlo:cmhi].astype(np.uint8)

    rm384 = ((np.arange(CANVAS_ROWS) + R0 - 42 >= 0)
             & (np.arange(CANVAS_ROWS) + R0 - 42 < W384)).astype(np.float32)
    rm192 = ((np.arange(ROWS192["A2"]) + r0 - 20 >= 0)
             & (np.arange(ROWS192["A2"]) + r0 - 20 < W192)).astype(np.float32)

    im = {
        "x1col": x1col.astype(NPBF16),
        "m_u8": np.broadcast_to(mbuf, (128,) + mbuf.shape).copy(),
        "rm384": np.broadcast_to(rm384, (128, CANVAS_ROWS)).astype(NPBF16),
        "rm192": np.broadcast_to(rm192, (128, ROWS192["A2"])).astype(NPBF16),
    }
    w1 = np.asarray(inputs["w1"], np.float32)  # [64, 3, 3, 3]
    w1r = np.empty((27, 64), np.float32)
    for t, (a, bb) in enumerate(TAPS):
        w1r[3 * t : 3 * t + 3] = w1[:, :, a + 1, bb + 1].T
    im["w1"] = w1r.astype(NPBF16)
    for i in range(2, 11):
        w = np.asarray(inputs[f"w{i}"], np.float32)  # [O, I, 3, 3]
        im[f"w{i}"] = np.ascontiguousarray(
            w.transpose(1, 2, 3, 0).reshape(w.shape[1], 9, w.shape[0])
        ).astype(NPBF16)
    for i in range(1, 11):
        bv = np.asarray(inputs[f"b{i}"], np.float32)
        im[f"b{i}"] = np.ascontiguousarray(bv.reshape(-1, min(bv.size, 128)).T)
    return im


_NC_CACHE = []


def _get_program():
    if not _NC_CACHE:
        _NC_CACHE.append(build_program())
    return _NC_CACHE[0]


def kernel(**inputs):
    nc = _get_program()
    in_maps = [make_core_inputs(inputs, c) for c in range(8)]
    res = run_bass_kernel_spmd(nc, in_maps, list(range(8)))
    out = np.empty((2, 512, W192, W192), np.float32)
    for c in range(8):
        b, s = c // 4, c % 4
        out[b, :, 48 * s : 48 * s + 48, :] = res.results[c]["out"]
    return out


# revision 13
# speedup vs baseline: 1.6338x; 1.0866x over previous
"""Trainium2 Bass kernel for DenseFeatureExtractionModule (irregular-pooled VGG).

Sharding: 8 cores = 2 images x 4 row-strips of the 192-grid output (48 rows
each). Each core receives its input strip with enough halo rows to compute
all 10 conv layers locally (no inter-core communication). Out-of-image halo
rows are kept at zero through the layer stack by multiplying edge-band rows
with a per-core row-validity mask, which reproduces SAME-conv zero padding.

V2: all activations + weights in bf16 (fp32 PSUM accumulation). Graph-conv
layers (5-10) use a gather-select path: per (cin-tile, tap) one bf16
tensor_copy (4x DVE mode) of the dilation-1 shifted window plus one
copy_predicated overlay of the dilation-s window, then a single set of
matmuls — halving PE work vs dual-dilation and cutting DVE traffic ~3.5x
vs the V1 gather. bf16 weights enable PE fast-weight-load.
"""

import numpy as np
import ml_dtypes

import concourse.bacc as bacc
import concourse.bass as bass
import concourse.mybir as mybir
import concourse.tile as tile
from concourse.bass_utils import run_bass_kernel_spmd

F32 = mybir.dt.float32
BF16 = mybir.dt.bfloat16
U8 = mybir.dt.uint8
RELU = mybir.ActivationFunctionType.Relu
MAX = mybir.AluOpType.max
NPBF16 = ml_dtypes.bfloat16

W192 = 192
W384 = 384
PAD = 4  # pad columns for all 192-grid buffers
WP = W192 + 2 * PAD  # 200
A1_WP = W384 + 2  # 386, pad 1

# buffer row counts (per core strips, incl. halo)
CANVAS_ROWS = 180  # batch strip on 384 grid (96 + 2*42)
A1_ROWS = 178
ROWS192 = {"A2": 88, "A3": 86, "A4": 84, "A5": 80, "A6": 76, "A7": 72,
           "A8": 64, "A9": 56, "OUT": 48}
HALO192 = {"A2": 20, "A3": 19, "A4": 18, "A5": 16, "A6": 14, "A7": 12,
           "A8": 8, "A9": 4, "OUT": 0}

_CHANS = [(3, 64), (64, 64), (64, 128), (128, 128), (128, 256),
          (256, 256), (256, 256), (256, 512), (512, 512), (512, 512)]

# (src, dst, Cin, Cout, dils, pool_s, widx) for layers 3..10
LAYERS = [
    ("A2", "A3", 64, 128, (1,), None, 3),
    ("A3", "A4", 128, 128, (1,), 2, 4),
    ("A4", "A5", 128, 256, (1, 2), None, 5),
    ("A5", "A6", 256, 256, (1, 2), None, 6),
    ("A6", "A7", 256, 256, (1, 2), 4, 7),
    ("A7", "A8", 256, 512, (1, 4), None, 8),
    ("A8", "A9", 512, 512, (1, 4), None, 9),
    ("A9", "OUT", 512, 512, (1, 4), None, 10),
]
RB = 8  # output rows per input-tile block (dense 192-grid layers)
GB = 4  # output rows per block in the gather path

TAPS = [(a, b) for a in (-1, 0, 1) for b in (-1, 0, 1)]


def _emit_zero_pads(nc, tc, pools, bufs):
    """Zero every DRAM scratch buffer with large contiguous DMAs (the layer
    interiors get overwritten; only the pad columns need to be zero, but
    full-row zeroing is ~50x cheaper per byte than column-strided writes)."""
    ZCHUNK = 4096  # free elems per partition per DMA
    zt = pools["const"].tile([128, ZCHUNK], BF16)
    nc.gpsimd.memset(zt[:], 0.0)
    for name, dram in bufs.items():
        if name in ("X1", "OUT"):
            continue
        C, rows, wp = dram.shape
        total = rows * wp
        for ci in range((C + 127) // 128):
            p = min(128, C - ci * 128)
            flat = dram[ci * 128 : ci * 128 + p].rearrange("p r c -> p (r c)")
            for o in range(0, total, ZCHUNK):
                n = min(ZCHUNK, total - o)
                nc.sync.dma_start(flat[:, o : o + n], zt[:p, :n])


def _emit_l1(nc, tc, pools, bufs, params):
    """L1: 1x1 conv over im2col'd input (K=27, M=64), 384 grid."""
    x1, a1 = bufs["X1"], bufs["A1"]
    wsb = pools["const"].tile([27, 64], BF16)
    nc.sync.dma_start(wsb[:], params["w1"][:])
    bsb = pools["const"].tile([64, 1], F32)
    nc.sync.dma_start(bsb[:], params["b1"][:])
    rm = pools["const"].tile([128, CANVAS_ROWS], BF16)
    nc.sync.dma_start(rm[:], params["rm384"][:])

    RB1 = 16
    with tc.tile_pool(name="in1", bufs=2) as p_in1, \
         tc.tile_pool(name="out1", bufs=4) as p_out1:
     for i0 in range(0, A1_ROWS, RB1):
        nr = min(RB1, A1_ROWS - i0)
        xt = p_in1.tile([27, RB1, W384], BF16, tag="x1t")
        nc.sync.dma_start(xt[:, :nr, :], x1[:, i0 : i0 + nr, :])
        for j in range(nr):
            i = i0 + j
            ps = pools["psum"].tile([64, W384], F32, tag="ps")
            nc.tensor.matmul(ps[:], wsb[:], xt[:, j, :], start=True, stop=True)
            ot = p_out1.tile([64, W384], BF16, tag="o1")
            nc.scalar.activation(ot[:], ps[:], RELU, bias=bsb[:])
            if i < 41 or i >= A1_ROWS - 41:
                nc.gpsimd.tensor_mul(
                    ot[:], ot[:],
                    rm[0:64, i + 1 : i + 2].broadcast_to([64, W384]))
            nc.sync.dma_start(a1[:, i, 1 : 1 + W384], ot[:])


def _emit_l2(nc, tc, pools, bufs, params):
    """L2 conv (64->64, 384 grid) fused with 2x2 maxpool -> A2 (192 grid)."""
    a1, a2 = bufs["A1"], bufs["A2"]
    wsb = pools["const"].tile([64, 9, 64], BF16)
    nc.sync.dma_start(wsb[:], params["w2"][:])
    bsb = pools["const"].tile([64, 1], F32)
    nc.sync.dma_start(bsb[:], params["b2"][:])
    rm = pools["rm192"]

    QB = 8  # A2 rows per block
    with tc.tile_pool(name="in2", bufs=2) as p_in2, \
         tc.tile_pool(name="out2", bufs=4) as p_out2:
     for q0 in range(0, ROWS192["A2"], QB):
        nq = min(QB, ROWS192["A2"] - q0)
        xt = p_in2.tile([64, 2 * QB + 2, A1_WP], BF16, tag="x2t")
        nc.sync.dma_start(xt[:, : 2 * nq + 2, :],
                          a1[:, 2 * q0 : 2 * q0 + 2 * nq + 2, :])
        for q in range(q0, q0 + nq):
            o2 = p_out2.tile([64, 2, W384], BF16, tag="o2")
            for r in range(2):
                ps = pools["psum"].tile([64, W384], F32, tag="ps")
                for ti, (a, b) in enumerate(TAPS):
                    rhs = xt[:, 2 * (q - q0) + r + 1 + a, 1 + b : 1 + b + W384]
                    nc.tensor.matmul(ps[:], wsb[:, ti, :], rhs,
                                     start=(ti == 0), stop=(ti == 8))
                nc.scalar.activation(o2[:, r, :], ps[:], RELU, bias=bsb[:])
            o2v = o2[:].rearrange("p r (c t) -> p r c t", t=2)
            cm = p_out2.tile([64, 2, W192], BF16, tag="cm2")
            nc.vector.tensor_tensor(cm[:, 0, :], o2v[:, 0, :, 0], o2v[:, 0, :, 1], MAX)
            nc.vector.tensor_tensor(cm[:, 1, :], o2v[:, 1, :, 0], o2v[:, 1, :, 1], MAX)
            mp = p_out2.tile([64, W192], BF16, tag="mp2")
            nc.vector.tensor_tensor(mp[:], cm[:, 0, :], cm[:, 1, :], MAX)
            if q < 20 or q >= ROWS192["A2"] - 20:
                nc.gpsimd.tensor_mul(
                    mp[:], mp[:], rm[0:64, q : q + 1].broadcast_to([64, W192]))
            nc.sync.dma_start(a2[:, q, PAD : PAD + W192], mp[:])


def _emit_dense192(nc, tc, pools, bufs, params, src, dst, cin, cout, dils,
                   pool_s, widx):
    """Dense 192-grid conv layer (single dilation), optional fused pool."""
    sdram, ddram = bufs[src], bufs[dst]
    rows_out = ROWS192[dst]
    h_out = HALO192[dst]
    nci = (cin + 127) // 128
    nco = (cout + 127) // 128
    off = 20 - h_out
    rm = pools["rm192"]
    msb = pools["m_u8"]

    wts = []
    for ci in range(nci):
        p = min(128, cin - ci * 128)
        wt = pools["wres"].tile([p, 9, cout], BF16, name=f"w{widx}_{ci}",
                                tag=f"wres_{ci}")
        nc.sync.dma_start(wt[:], params[f"w{widx}"][ci * 128 : ci * 128 + p])
        wts.append(wt)
    bsb = pools["const"].tile([min(cout, 128), nco], F32, name=f"bsb{widx}")
    nc.sync.dma_start(bsb[:], params[f"b{widx}"][:])

    grp = pool_s if pool_s else 2  # rows per output tile group
    with tc.tile_pool(name=f"xin{widx}", bufs=2) as p_xin:
     for j0 in range(0, rows_out, RB):
        rb = min(RB, rows_out - j0)
        xts = []
        for ci in range(nci):
            p = min(128, cin - ci * 128)
            xt = p_xin.tile([p, RB + 2, WP], BF16, tag=f"xin{ci}")
            nc.sync.dma_start(xt[:, : rb + 2, :],
                              sdram[ci * 128 : ci * 128 + p, j0 : j0 + rb + 2, :])
            xts.append(xt)
        for co in range(nco):
            pco = min(128, cout - co * 128)
            for g0 in range(0, rb, grp):
                tg = pools["oacc"].tile([pco, grp, W192], BF16, tag="oacc")
                for rp in range(grp // 2):
                    j = j0 + g0 + rp * 2
                    ps = pools["psum"].tile([pco, 2 * W192], F32, tag="ps")
                    for ci in range(nci):
                        for ti, (a, b) in enumerate(TAPS):
                            rhs = xts[ci][:, g0 + rp * 2 + 1 + a :
                                          g0 + rp * 2 + 1 + a + 2,
                                          PAD + b : PAD + b + W192]
                            nc.tensor.matmul(
                                ps[:],
                                wts[ci][:, ti, co * 128 : co * 128 + pco],
                                rhs,
                                start=(ci == 0 and ti == 0),
                                stop=(ci == nci - 1 and ti == 8))
                    t1 = tg[:, rp * 2 : rp * 2 + 2, :]
                    psv = ps[:].rearrange("p (r w) -> p r w", w=W192)
                    nc.scalar.activation(t1, psv, RELU, bias=bsb[:pco, co : co + 1])
                    if j < h_out or j + 2 > rows_out - h_out:
                        nc.gpsimd.tensor_mul(
                            t1, t1,
                            rm[:pco, off + j : off + j + 2].unsqueeze(-1)
                            .broadcast_to([pco, 2, W192]))
                j = j0 + g0
                if pool_s == 2:
                    tv = tg[:].rearrange("p r (c t) -> p r c t", t=2)
                    cm = pools["pscr"].tile([pco, 2, W192 // 2], BF16, tag="pcm")
                    nc.vector.tensor_tensor(cm[:], tv[:, :, :, 0], tv[:, :, :, 1], MAX)
                    bm = pools["pscr"].tile([pco, W192 // 2], BF16, tag="pbm")
                    nc.vector.tensor_tensor(bm[:], cm[:, 0, :], cm[:, 1, :], MAX)
                    rep = pools["pscr"].tile([pco, 2, W192], BF16, tag="prep")
                    nc.vector.tensor_copy(
                        rep[:], bm[:].unsqueeze(1).unsqueeze(-1)
                        .broadcast_to([pco, 2, W192 // 2, 2]))
                    nc.vector.copy_predicated(
                        tg[:], msb[:pco, off + j : off + j + 2, :], rep[:])
                nc.sync.dma_start(
                    ddram[co * 128 : co * 128 + pco, j : j + grp,
                          PAD : PAD + W192], tg[:])


def _emit_gather192(nc, tc, pools, bufs, params, src, dst, cin, cout, dils,
                    pool_s, widx):
    """Graph-conv layer via gather-select: one bf16 copy (4x DVE mode) + one
    copy_predicated overlay per tap covering ALL cin-tiles (stacked along the
    free axis), then N<=512 matmul chains. Optional fused s=4 pool."""
    sdram, ddram = bufs[src], bufs[dst]
    rows_out = ROWS192[dst]
    h_out = HALO192[dst]
    s = dils[1]
    dm = s
    nci = (cin + 127) // 128
    nco = (cout + 127) // 128
    off = 20 - h_out
    act_dt = F32 if dst == "OUT" else BF16
    rm = pools["rm192"]
    msb = pools["m_u8"]
    dst_c0 = 0 if dst == "OUT" else PAD
    GBl = 8 if nco <= 2 else 4  # psum banks: nco * ceil(GBl*192/512) <= 8
    S = GBl + 2 * dm  # xin rows per cin-tile slot

    wts = []
    for ci in range(nci):
        wt = pools["wres"].tile([128, 9, cout], BF16, name=f"w{widx}_{ci}",
                                tag=f"wres_{ci}")
        nc.sync.dma_start(wt[:], params[f"w{widx}"][ci * 128 : ci * 128 + 128])
        wts.append(wt)
    bsb = pools["const"].tile([min(cout, 128), nco], F32, name=f"bsb{widx}")
    nc.sync.dma_start(bsb[:], params[f"b{widx}"][:])

    with tc.tile_pool(name=f"xin{widx}", bufs=2) as p_xin, \
         tc.tile_pool(name=f"gp{widx}", bufs=6) as p_g:
        for j0 in range(0, rows_out, GBl):
            gb = min(GBl, rows_out - j0)
            ncol = gb * W192
            bounds = [(k * 512, min(ncol, (k + 1) * 512))
                      for k in range((ncol + 511) // 512)]
            xt = p_xin.tile([128, nci * S, WP], BF16, tag="xin")
            for ci in range(nci):
                nc.sync.dma_start(
                    xt[:, ci * S : ci * S + gb + 2 * dm, :],
                    sdram[ci * 128 : ci * 128 + 128, j0 : j0 + gb + 2 * dm, :])
            xv = xt[:].rearrange("p (n r) c -> p n r c", n=nci)
            pss = [[pools["psum"].tile([128, hi - lo], F32, tag="ps",
                                       name=f"ps{widx}_{j0}_{co}_{k}")
                    for k, (lo, hi) in enumerate(bounds)] for co in range(nco)]
            for ti, (a, b) in enumerate(TAPS):
                g = p_g.tile([128, nci, GBl, W192], BF16, tag="g3")
                nc.vector.tensor_copy(
                    g[:, :, :gb, :], xv[:, :, dm + a : dm + a + gb,
                                        PAD + b : PAD + b + W192])
                if not (a == 0 and b == 0):
                    nc.vector.copy_predicated(
                        g[:, :, :gb, :],
                        msb[:, off + j0 : off + j0 + gb, :].unsqueeze(1)
                           .broadcast_to([128, nci, gb, W192]),
                        xv[:, :, dm + a * s : dm + a * s + gb,
                           PAD + b * s : PAD + b * s + W192])
                gf = g[:].rearrange("p n r w -> p n (r w)")
                for ci in range(nci):
                    for co in range(nco):
                        pco = min(128, cout - co * 128)
                        for k, (lo, hi) in enumerate(bounds):
                            nc.tensor.matmul(
                                pss[co][k][:pco, :],
                                wts[ci][:, ti, co * 128 : co * 128 + pco],
                                gf[:, ci, lo:hi],
                                start=(ti == 0 and ci == 0),
                                stop=(ti == 8 and ci == nci - 1))
            for co in range(nco):
                pco = min(128, cout - co * 128)
                tg = pools["oacc"].tile([pco, GBl, W192], act_dt, tag="oacc")
                tgf = tg[:].rearrange("p r w -> p (r w)")
                for k, (lo, hi) in enumerate(bounds):
                    nc.scalar.activation(tgf[:, lo:hi], pss[co][k][:pco, :],
                                         RELU, bias=bsb[:pco, co : co + 1])
                if j0 < h_out or j0 + gb > rows_out - h_out:
                    nc.gpsimd.tensor_mul(
                        tg[:, :gb, :], tg[:, :gb, :],
                        rm[:pco, off + j0 : off + j0 + gb].unsqueeze(-1)
                        .broadcast_to([pco, gb, W192]))
                if pool_s == 4:
                    for q0 in range(0, gb, 4):
                        tq = tg[:, q0 : q0 + 4, :]
                        tv = tq.rearrange("p r (c t) -> p r c t", t=4)
                        c1 = pools["pscr"].tile([pco, 4, W192 // 4], BF16, tag="pc1")
                        c2 = pools["pscr"].tile([pco, 4, W192 // 4], BF16, tag="pc2")
                        nc.vector.tensor_tensor(c1[:], tv[:, :, :, 0], tv[:, :, :, 1], MAX)
                        nc.vector.tensor_tensor(c2[:], tv[:, :, :, 2], tv[:, :, :, 3], MAX)
                        nc.vector.tensor_tensor(c1[:], c1[:], c2[:], MAX)
                        r1 = pools["pscr"].tile([pco, W192 // 4], BF16, tag="pr1")
                        r2 = pools["pscr"].tile([pco, W192 // 4], BF16, tag="pr2")
                        nc.vector.tensor_tensor(r1[:], c1[:, 0, :], c1[:, 1, :], MAX)
                        nc.vector.tensor_tensor(r2[:], c1[:, 2, :], c1[:, 3, :], MAX)
                        nc.vector.tensor_tensor(r1[:], r1[:], r2[:], MAX)
                        rep = pools["pscr"].tile([pco, 4, W192], BF16, tag="prep4")
                        nc.vector.tensor_copy(
                            rep[:], r1[:].unsqueeze(1).unsqueeze(-1)
                            .broadcast_to([pco, 4, W192 // 4, 4]))
                        nc.vector.copy_predicated(
                            tq, msb[:pco, off + j0 + q0 : off + j0 + q0 + 4, :],
                            rep[:])
                nc.sync.dma_start(
                    ddram[co * 128 : co * 128 + pco, j0 : j0 + gb,
                          dst_c0 : dst_c0 + W192], tg[:, :gb, :])


def build_program():
    nc = bacc.Bacc()
    params = {}
    params["x1col"] = nc.declare_dram_parameter(
        "x1col", [27, A1_ROWS, W384], BF16, isOutput=False)
    params["w1"] = nc.declare_dram_parameter("w1", [27, 64], BF16, isOutput=False)
    for i, (ci, co) in enumerate(_CHANS):
        if i > 0:
            params[f"w{i + 1}"] = nc.declare_dram_parameter(
                f"w{i + 1}", [ci, 9, co], BF16, isOutput=False)
        params[f"b{i + 1}"] = nc.declare_dram_parameter(
            f"b{i + 1}", [min(co, 128), (co + 127) // 128], F32, isOutput=False)
    params["m_u8"] = nc.declare_dram_parameter(
        "m_u8", [128, ROWS192["A2"], W192], U8, isOutput=False)
    params["rm384"] = nc.declare_dram_parameter(
        "rm384", [128, CANVAS_ROWS], BF16, isOutput=False)
    params["rm192"] = nc.declare_dram_parameter(
        "rm192", [128, ROWS192["A2"]], BF16, isOutput=False)

    bufs = {"X1": params["x1col"]}
    bufs["A1"] = nc.dram_tensor("A1", [64, A1_ROWS, A1_WP], BF16)
    for name, cc in (("A2", 64), ("A3", 128), ("A4", 128), ("A5", 256),
                     ("A6", 256), ("A7", 256), ("A8", 512), ("A9", 512)):
        bufs[name] = nc.dram_tensor(name, [cc, ROWS192[name], WP], BF16)
    bufs["OUT"] = nc.declare_dram_parameter(
        "out", [512, ROWS192["OUT"], W192], F32, isOutput=True)

    with tile.TileContext(nc) as tc:
        from contextlib import ExitStack
        with ExitStack() as ctx:
            pools = {}
            for name, kw in (
                ("const", dict(bufs=1)),
                ("oacc", dict(bufs=6)),
                ("pscr", dict(bufs=2)),
                ("psum", dict(bufs=8, space="PSUM")),
            ):
                pools[name] = ctx.enter_context(tc.tile_pool(name=name, **kw))
            # resident masks
            pools["m_u8"] = pools["const"].tile([128, ROWS192["A2"], W192], U8,
                                                name="m_u8_t", tag="m_u8")
            nc.sync.dma_start(pools["m_u8"][:], params["m_u8"][:])
            pools["rm192"] = pools["const"].tile([128, ROWS192["A2"]], BF16,
                                                 name="rm192_t", tag="rm192")
            nc.sync.dma_start(pools["rm192"][:], params["rm192"][:])

            _emit_zero_pads(nc, tc, pools, bufs)
            _emit_l1(nc, tc, pools, bufs, params)
            _emit_l2(nc, tc, pools, bufs, params)
            with tc.tile_pool(name="wres", bufs=2) as p_wres:
                pools["wres"] = p_wres
                for lay in LAYERS:
                    if len(lay[4]) == 1:
                        _emit_dense192(nc, tc, pools, bufs, params, *lay)
                    else:
                        _emit_gather192(nc, tc, pools, bufs, params, *lay)
    nc.compile()
    return nc


# ---------------------------------------------------------------- host side

def _upsample_mask(m48):
    return np.repeat(np.repeat(m48, 4, axis=0), 4, axis=1)


def make_core_inputs(inputs, core):
    b, s = core // 4, core % 4
    r0, R0 = 48 * s, 96 * s
    x = np.asarray(inputs["batch"][b], np.float32)  # [3, 384, 384]

    canvas = np.zeros((3, CANVAS_ROWS, W384 + 2), np.float32)
    lo, hi = R0 - 42, R0 + 138
    clo, chi = max(lo, 0), min(hi, W384)
    canvas[:, clo - lo : chi - lo, 1 : 1 + W384] = x[:, clo:chi, :]

    x1col = np.empty((27, A1_ROWS, W384), np.float32)
    for t, (a, bb) in enumerate(TAPS):
        x1col[3 * t : 3 * t + 3] = canvas[:, 1 + a : 1 + a + A1_ROWS,
                                          1 + bb : 1 + bb + W384]

    m192 = _upsample_mask(np.asarray(inputs["pooling_mask"][b, 0]))  # [192,192]
    mbuf = np.zeros((ROWS192["A2"], W192), np.uint8)
    mlo, mhi = r0 - 20, r0 + 68
    cmlo, cmhi = max(mlo, 0), min(mhi, W192)
    mbuf[cmlo - mlo : cmhi - mlo] = m192[cmlo:cmhi].astype(np.uint8)

    rm384 = ((np.arange(CANVAS_ROWS) + R0 - 42 >= 0)
             & (np.arange(CANVAS_ROWS) + R0 - 42 < W384)).astype(np.float32)
    rm192 = ((np.arange(ROWS192["A2"]) + r0 - 20 >= 0)
             & (np.arange(ROWS192["A2"]) + r0 - 20 < W192)).astype(np.float32)

    im = {
        "x1col": x1col.astype(NPBF16),
        "m_u8": np.broadcast_to(mbuf, (128,) + mbuf.shape).copy(),
        "rm384": np.broadcast_to(rm384, (128, CANVAS_ROWS)).astype(NPBF16),
        "rm192": np.broadcast_to(rm192, (128, ROWS192["A2"])).astype(NPBF16),
    }
    w1 = np.asarray(inputs["w1"], np.float32)  # [64, 3, 3, 3]
    w1r = np.empty((27, 64), np.float32)
    for t, (a, bb) in enumerate(TAPS):
        w1r[3 * t : 3 * t + 3] = w1[:, :, a + 1, bb + 1].T
    im["w1"] = w1r.astype(NPBF16)
    for i in range(2, 11):
        w = np.asarray(inputs[f"w{i}"], np.float32)  # [O, I, 3, 3]
        im[f"w{i}"] = np.ascontiguousarray(
            w.transpose(1, 2, 3, 0).reshape(w.shape[1], 9, w.shape[0])
        ).astype(NPBF16)
    for i in range(1, 11):
        bv = np.asarray(inputs[f"b{i}"], np.float32)
        im[f"b{i}"] = np.ascontiguousarray(bv.reshape(-1, min(bv.size, 128)).T)
    return im


_NC_CACHE = []


def _get_program():
    if not _NC_CACHE:
        _NC_CACHE.append(build_program())
    return _NC_CACHE[0]


def kernel(**inputs):
    nc = _get_program()
    in_maps = [make_core_inputs(inputs, c) for c in range(8)]
    res = run_bass_kernel_spmd(nc, in_maps, list(range(8)))
    out = np.empty((2, 512, W192, W192), np.float32)
    for c in range(8):
        b, s = c // 4, c % 4
        out[b, :, 48 * s : 48 * s + 48, :] = res.results[c]["out"]
    return out


# revision 17
# speedup vs baseline: 1.6977x; 1.0391x over previous
"""Trainium2 Bass kernel for DenseFeatureExtractionModule (irregular-pooled VGG).

Sharding: 8 cores = 2 images x 4 row-strips of the 192-grid output (48 rows
each). Each core receives its input strip with enough halo rows to compute
all 10 conv layers locally (no inter-core communication). Out-of-image halo
rows are kept at zero through the layer stack by multiplying edge-band rows
with a per-core row-validity mask, which reproduces SAME-conv zero padding.

V2: all activations + weights in bf16 (fp32 PSUM accumulation). Graph-conv
layers (5-10) use a gather-select path: per (cin-tile, tap) one bf16
tensor_copy (4x DVE mode) of the dilation-1 shifted window plus one
copy_predicated overlay of the dilation-s window, then a single set of
matmuls — halving PE work vs dual-dilation and cutting DVE traffic ~3.5x
vs the V1 gather. bf16 weights enable PE fast-weight-load.
"""

import numpy as np
import ml_dtypes

import concourse.bacc as bacc
import concourse.bass as bass
import concourse.mybir as mybir
import concourse.tile as tile
from concourse.bass_utils import run_bass_kernel_spmd

F32 = mybir.dt.float32
BF16 = mybir.dt.bfloat16
U8 = mybir.dt.uint8
RELU = mybir.ActivationFunctionType.Relu
MAX = mybir.AluOpType.max
NPBF16 = ml_dtypes.bfloat16

W192 = 192
W384 = 384
PAD = 4  # pad columns for all 192-grid buffers
WP = W192 + 2 * PAD  # 200
A1_WP = W384 + 2  # 386, pad 1

# buffer row counts (per core strips, incl. halo)
CANVAS_ROWS = 180  # batch strip on 384 grid (96 + 2*42)
A1_ROWS = 178
ROWS192 = {"A2": 88, "A3": 86, "A4": 84, "A5": 80, "A6": 76, "A7": 72,
           "A8": 64, "A9": 56, "OUT": 48}
HALO192 = {"A2": 20, "A3": 19, "A4": 18, "A5": 16, "A6": 14, "A7": 12,
           "A8": 8, "A9": 4, "OUT": 0}

_CHANS = [(3, 64), (64, 64), (64, 128), (128, 128), (128, 256),
          (256, 256), (256, 256), (256, 512), (512, 512), (512, 512)]

# (src, dst, Cin, Cout, dils, pool_s, widx) for layers 3..10
LAYERS = [
    ("A2", "A3", 64, 128, (1,), None, 3),
    ("A3", "A4", 128, 128, (1,), 2, 4),
    ("A4", "A5", 128, 256, (1, 2), None, 5),
    ("A5", "A6", 256, 256, (1, 2), None, 6),
    ("A6", "A7", 256, 256, (1, 2), 4, 7),
    ("A7", "A8", 256, 512, (1, 4), None, 8),
    ("A8", "A9", 512, 512, (1, 4), None, 9),
    ("A9", "OUT", 512, 512, (1, 4), None, 10),
]
RB = 8  # output rows per input-tile block (dense 192-grid layers)
GB = 4  # output rows per block in the gather path

TAPS = [(a, b) for a in (-1, 0, 1) for b in (-1, 0, 1)]


ZCHUNK = 4096  # free elems per partition per zero-fill DMA


def _emit_zero_buf(nc, pools, dram):
    """Zero one DRAM scratch buffer with large contiguous DMAs (the layer
    interior gets overwritten; only the pad columns need to be zero, but
    full-row zeroing is ~50x cheaper per byte than column-strided writes).
    Emitted just before the producing layer so the DMAs overlap its compute
    instead of clogging the queues at kernel start."""
    C, rows, wp = dram.shape
    total = rows * wp
    zt = pools["zt"]
    for ci in range((C + 127) // 128):
        p = min(128, C - ci * 128)
        flat = dram[ci * 128 : ci * 128 + p].rearrange("p r c -> p (r c)")
        for o in range(0, total, ZCHUNK):
            n = min(ZCHUNK, total - o)
            nc.sync.dma_start(flat[:, o : o + n], zt[:p, :n])


def _emit_l1(nc, tc, pools, bufs, params):
    """L1: 1x1 conv over im2col'd input (K=27, M=64), 384 grid."""
    x1, a1 = bufs["X1"], bufs["A1"]
    _emit_zero_buf(nc, pools, a1)
    wsb = pools["const"].tile([27, 64], BF16)
    nc.sync.dma_start(wsb[:], params["w1"][:])
    bsb = pools["const"].tile([64, 1], F32)
    nc.sync.dma_start(bsb[:], params["b1"][:])
    rm = pools["const"].tile([128, CANVAS_ROWS], BF16)
    nc.sync.dma_start(rm[:], params["rm384"][:])

    RB1 = 16
    with tc.tile_pool(name="in1", bufs=2) as p_in1, \
         tc.tile_pool(name="out1", bufs=4) as p_out1:
     for i0 in range(0, A1_ROWS, RB1):
        nr = min(RB1, A1_ROWS - i0)
        xt = p_in1.tile([27, RB1, W384], BF16, tag="x1t")
        nc.sync.dma_start(xt[:, :nr, :], x1[:, i0 : i0 + nr, :])
        for j in range(nr):
            i = i0 + j
            ps = pools["psum"].tile([64, W384], F32, tag="ps")
            nc.tensor.matmul(ps[:], wsb[:], xt[:, j, :], start=True, stop=True)
            ot = p_out1.tile([64, W384], BF16, tag="o1")
            nc.scalar.activation(ot[:], ps[:], RELU, bias=bsb[:])
            if i < 41 or i >= A1_ROWS - 41:
                nc.gpsimd.tensor_mul(
                    ot[:], ot[:],
                    rm[0:64, i + 1 : i + 2].broadcast_to([64, W384]))
            nc.sync.dma_start(a1[:, i, 1 : 1 + W384], ot[:])


def _emit_l2(nc, tc, pools, bufs, params):
    """L2 conv (64->64, 384 grid) fused with 2x2 maxpool -> A2 (192 grid)."""
    a1, a2 = bufs["A1"], bufs["A2"]
    _emit_zero_buf(nc, pools, a2)
    wsb = pools["const"].tile([64, 9, 64], BF16)
    nc.sync.dma_start(wsb[:], params["w2"][:])
    bsb = pools["const"].tile([64, 1], F32)
    nc.sync.dma_start(bsb[:], params["b2"][:])
    rm = pools["rm192"]

    QB = 8  # A2 rows per block
    with tc.tile_pool(name="in2", bufs=2) as p_in2, \
         tc.tile_pool(name="out2", bufs=4) as p_out2:
     for q0 in range(0, ROWS192["A2"], QB):
        nq = min(QB, ROWS192["A2"] - q0)
        xt = p_in2.tile([64, 2 * QB + 2, A1_WP], BF16, tag="x2t")
        nc.sync.dma_start(xt[:, : 2 * nq + 2, :],
                          a1[:, 2 * q0 : 2 * q0 + 2 * nq + 2, :])
        for q in range(q0, q0 + nq):
            o2 = p_out2.tile([64, 2, W384], BF16, tag="o2")
            for r in range(2):
                ps = pools["psum"].tile([64, W384], F32, tag="ps")
                for ti, (a, b) in enumerate(TAPS):
                    rhs = xt[:, 2 * (q - q0) + r + 1 + a, 1 + b : 1 + b + W384]
                    nc.tensor.matmul(ps[:], wsb[:, ti, :], rhs,
                                     start=(ti == 0), stop=(ti == 8))
                nc.scalar.activation(o2[:, r, :], ps[:], RELU, bias=bsb[:])
            o2v = o2[:].rearrange("p r (c t) -> p r c t", t=2)
            cm = p_out2.tile([64, 2, W192], BF16, tag="cm2")
            nc.vector.tensor_tensor(cm[:, 0, :], o2v[:, 0, :, 0], o2v[:, 0, :, 1], MAX)
            nc.vector.tensor_tensor(cm[:, 1, :], o2v[:, 1, :, 0], o2v[:, 1, :, 1], MAX)
            mp = p_out2.tile([64, W192], BF16, tag="mp2")
            nc.vector.tensor_tensor(mp[:], cm[:, 0, :], cm[:, 1, :], MAX)
            if q < 20 or q >= ROWS192["A2"] - 20:
                nc.gpsimd.tensor_mul(
                    mp[:], mp[:], rm[0:64, q : q + 1].broadcast_to([64, W192]))
            nc.sync.dma_start(a2[:, q, PAD : PAD + W192], mp[:])


def _emit_dense192(nc, tc, pools, bufs, params, src, dst, cin, cout, dils,
                   pool_s, widx):
    """Dense 192-grid conv layer (single dilation), optional fused pool."""
    sdram, ddram = bufs[src], bufs[dst]
    if dst != "OUT":
        _emit_zero_buf(nc, pools, ddram)
    rows_out = ROWS192[dst]
    h_out = HALO192[dst]
    nci = (cin + 127) // 128
    nco = (cout + 127) // 128
    off = 20 - h_out
    rm = pools["rm192"]
    msb = pools["m_u8"]

    wts = []
    for ci in range(nci):
        p = min(128, cin - ci * 128)
        wt = pools["wres"].tile([p, 9, cout], BF16, name=f"w{widx}_{ci}",
                                tag=f"wres_{ci}")
        nc.sync.dma_start(wt[:], params[f"w{widx}"][ci * 128 : ci * 128 + p])
        wts.append(wt)
    bsb = pools["const"].tile([min(cout, 128), nco], F32, name=f"bsb{widx}")
    nc.sync.dma_start(bsb[:], params[f"b{widx}"][:])

    grp = pool_s if pool_s else 2  # rows per output tile group
    with tc.tile_pool(name=f"xin{widx}", bufs=2) as p_xin:
     for j0 in range(0, rows_out, RB):
        rb = min(RB, rows_out - j0)
        xts = []
        for ci in range(nci):
            p = min(128, cin - ci * 128)
            xt = p_xin.tile([p, RB + 2, WP], BF16, tag=f"xin{ci}")
            nc.sync.dma_start(xt[:, : rb + 2, :],
                              sdram[ci * 128 : ci * 128 + p, j0 : j0 + rb + 2, :])
            xts.append(xt)
        for co in range(nco):
            pco = min(128, cout - co * 128)
            for g0 in range(0, rb, grp):
                tg = pools["oacc"].tile([pco, grp, W192], BF16, tag="oacc")
                for rp in range(grp // 2):
                    j = j0 + g0 + rp * 2
                    ps = pools["psum"].tile([pco, 2 * W192], F32, tag="ps")
                    for ci in range(nci):
                        for ti, (a, b) in enumerate(TAPS):
                            rhs = xts[ci][:, g0 + rp * 2 + 1 + a :
                                          g0 + rp * 2 + 1 + a + 2,
                                          PAD + b : PAD + b + W192]
                            nc.tensor.matmul(
                                ps[:],
                                wts[ci][:, ti, co * 128 : co * 128 + pco],
                                rhs,
                                start=(ci == 0 and ti == 0),
                                stop=(ci == nci - 1 and ti == 8))
                    t1 = tg[:, rp * 2 : rp * 2 + 2, :]
                    psv = ps[:].rearrange("p (r w) -> p r w", w=W192)
                    nc.scalar.activation(t1, psv, RELU, bias=bsb[:pco, co : co + 1])
                    if j < h_out or j + 2 > rows_out - h_out:
                        nc.gpsimd.tensor_mul(
                            t1, t1,
                            rm[:pco, off + j : off + j + 2].unsqueeze(-1)
                            .broadcast_to([pco, 2, W192]))
                j = j0 + g0
                if pool_s == 2:
                    tv = tg[:].rearrange("p r (c t) -> p r c t", t=2)
                    cm = pools["pscr"].tile([pco, 2, W192 // 2], BF16, tag="pcm")
                    nc.vector.tensor_tensor(cm[:], tv[:, :, :, 0], tv[:, :, :, 1], MAX)
                    bm = pools["pscr"].tile([pco, W192 // 2], BF16, tag="pbm")
                    nc.vector.tensor_tensor(bm[:], cm[:, 0, :], cm[:, 1, :], MAX)
                    rep = pools["pscr"].tile([pco, 2, W192], BF16, tag="prep")
                    nc.vector.tensor_copy(
                        rep[:], bm[:].unsqueeze(1).unsqueeze(-1)
                        .broadcast_to([pco, 2, W192 // 2, 2]))
                    nc.vector.copy_predicated(
                        tg[:], msb[:pco, off + j : off + j + 2, :], rep[:])
                nc.sync.dma_start(
                    ddram[co * 128 : co * 128 + pco, j : j + grp,
                          PAD : PAD + W192], tg[:])


def _emit_gather192(nc, tc, pools, bufs, params, src, dst, cin, cout, dils,
                    pool_s, widx):
    """Graph-conv layer via gather-select: one bf16 copy (4x DVE mode) + one
    copy_predicated overlay per tap covering ALL cin-tiles (stacked along the
    free axis), then N<=512 matmul chains. Optional fused s=4 pool."""
    sdram, ddram = bufs[src], bufs[dst]
    if dst != "OUT":
        _emit_zero_buf(nc, pools, ddram)
    rows_out = ROWS192[dst]
    h_out = HALO192[dst]
    s = dils[1]
    dm = s
    nci = (cin + 127) // 128
    nco = (cout + 127) // 128
    off = 20 - h_out
    act_dt = F32 if dst == "OUT" else BF16
    rm = pools["rm192"]
    msb = pools["m_u8"]
    dst_c0 = 0 if dst == "OUT" else PAD
    GBl = 8 if nco <= 2 else 4  # psum banks: nco * ceil(GBl*192/512) <= 8
    S = GBl + 2 * dm  # xin rows per cin-tile slot

    wts = []
    for ci in range(nci):
        wt = pools["wres"].tile([128, 9, cout], BF16, name=f"w{widx}_{ci}",
                                tag=f"wres_{ci}")
        nc.sync.dma_start(wt[:], params[f"w{widx}"][ci * 128 : ci * 128 + 128])
        wts.append(wt)
    bsb = pools["const"].tile([min(cout, 128), nco], F32, name=f"bsb{widx}")
    nc.sync.dma_start(bsb[:], params[f"b{widx}"][:])

    with tc.tile_pool(name=f"xin{widx}", bufs=2) as p_xin, \
         tc.tile_pool(name=f"gp{widx}", bufs=6) as p_g:
        for j0 in range(0, rows_out, GBl):
            gb = min(GBl, rows_out - j0)
            ncol = gb * W192
            bounds = [(k * 512, min(ncol, (k + 1) * 512))
                      for k in range((ncol + 511) // 512)]
            xt = p_xin.tile([128, nci * S, WP], BF16, tag="xin")
            for ci in range(nci):
                nc.sync.dma_start(
                    xt[:, ci * S : ci * S + gb + 2 * dm, :],
                    sdram[ci * 128 : ci * 128 + 128, j0 : j0 + gb + 2 * dm, :])
            xv = xt[:].rearrange("p (n r) c -> p n r c", n=nci)
            pss = [[pools["psum"].tile([128, hi - lo], F32, tag="ps",
                                       name=f"ps{widx}_{j0}_{co}_{k}")
                    for k, (lo, hi) in enumerate(bounds)] for co in range(nco)]
            for ti, (a, b) in enumerate(TAPS):
                g = p_g.tile([128, nci, GBl, W192], BF16, tag="g3")
                nc.vector.tensor_copy(
                    g[:, :, :gb, :], xv[:, :, dm + a : dm + a + gb,
                                        PAD + b : PAD + b + W192])
                if not (a == 0 and b == 0):
                    nc.vector.copy_predicated(
                        g[:, :, :gb, :],
                        msb[:, off + j0 : off + j0 + gb, :].unsqueeze(1)
                           .broadcast_to([128, nci, gb, W192]),
                        xv[:, :, dm + a * s : dm + a * s + gb,
                           PAD + b * s : PAD + b * s + W192])
                gf = g[:].rearrange("p n r w -> p n (r w)")
                for ci in range(nci):
                    for co in range(nco):
                        pco = min(128, cout - co * 128)
                        for k, (lo, hi) in enumerate(bounds):
                            nc.tensor.matmul(
                                pss[co][k][:pco, :],
                                wts[ci][:, ti, co * 128 : co * 128 + pco],
                                gf[:, ci, lo:hi],
                                start=(ti == 0 and ci == 0),
                                stop=(ti == 8 and ci == nci - 1))
            for co in range(nco):
                pco = min(128, cout - co * 128)
                tg = pools["oacc"].tile([pco, GBl, W192], act_dt, tag="oacc")
                tgf = tg[:].rearrange("p r w -> p (r w)")
                for k, (lo, hi) in enumerate(bounds):
                    nc.scalar.activation(tgf[:, lo:hi], pss[co][k][:pco, :],
                                         RELU, bias=bsb[:pco, co : co + 1])
                if j0 < h_out or j0 + gb > rows_out - h_out:
                    nc.gpsimd.tensor_mul(
                        tg[:, :gb, :], tg[:, :gb, :],
                        rm[:pco, off + j0 : off + j0 + gb].unsqueeze(-1)
                        .broadcast_to([pco, gb, W192]))
                if pool_s == 4:
                    for q0 in range(0, gb, 4):
                        tq = tg[:, q0 : q0 + 4, :]
                        tv = tq.rearrange("p r (c t) -> p r c t", t=4)
                        c1 = pools["pscr"].tile([pco, 4, W192 // 4], BF16, tag="pc1")
                        c2 = pools["pscr"].tile([pco, 4, W192 // 4], BF16, tag="pc2")
                        nc.vector.tensor_tensor(c1[:], tv[:, :, :, 0], tv[:, :, :, 1], MAX)
                        nc.vector.tensor_tensor(c2[:], tv[:, :, :, 2], tv[:, :, :, 3], MAX)
                        nc.vector.tensor_tensor(c1[:], c1[:], c2[:], MAX)
                        r1 = pools["pscr"].tile([pco, W192 // 4], BF16, tag="pr1")
                        r2 = pools["pscr"].tile([pco, W192 // 4], BF16, tag="pr2")
                        nc.vector.tensor_tensor(r1[:], c1[:, 0, :], c1[:, 1, :], MAX)
                        nc.vector.tensor_tensor(r2[:], c1[:, 2, :], c1[:, 3, :], MAX)
                        nc.vector.tensor_tensor(r1[:], r1[:], r2[:], MAX)
                        rep = pools["pscr"].tile([pco, 4, W192], BF16, tag="prep4")
                        nc.vector.tensor_copy(
                            rep[:], r1[:].unsqueeze(1).unsqueeze(-1)
                            .broadcast_to([pco, 4, W192 // 4, 4]))
                        nc.vector.copy_predicated(
                            tq, msb[:pco, off + j0 + q0 : off + j0 + q0 + 4, :],
                            rep[:])
                nc.sync.dma_start(
                    ddram[co * 128 : co * 128 + pco, j0 : j0 + gb,
                          dst_c0 : dst_c0 + W192], tg[:, :gb, :])


def build_program():
    nc = bacc.Bacc()
    params = {}
    params["x1col"] = nc.declare_dram_parameter(
        "x1col", [27, A1_ROWS, W384], BF16, isOutput=False)
    params["w1"] = nc.declare_dram_parameter("w1", [27, 64], BF16, isOutput=False)
    for i, (ci, co) in enumerate(_CHANS):
        if i > 0:
            params[f"w{i + 1}"] = nc.declare_dram_parameter(
                f"w{i + 1}", [ci, 9, co], BF16, isOutput=False)
        params[f"b{i + 1}"] = nc.declare_dram_parameter(
            f"b{i + 1}", [min(co, 128), (co + 127) // 128], F32, isOutput=False)
    params["m_u8"] = nc.declare_dram_parameter(
        "m_u8", [128, ROWS192["A2"], W192], U8, isOutput=False)
    params["rm384"] = nc.declare_dram_parameter(
        "rm384", [128, CANVAS_ROWS], BF16, isOutput=False)
    params["rm192"] = nc.declare_dram_parameter(
        "rm192", [128, ROWS192["A2"]], BF16, isOutput=False)

    bufs = {"X1": params["x1col"]}
    bufs["A1"] = nc.dram_tensor("A1", [64, A1_ROWS, A1_WP], BF16)
    for name, cc in (("A2", 64), ("A3", 128), ("A4", 128), ("A5", 256),
                     ("A6", 256), ("A7", 256), ("A8", 512), ("A9", 512)):
        bufs[name] = nc.dram_tensor(name, [cc, ROWS192[name], WP], BF16)
    bufs["OUT"] = nc.declare_dram_parameter(
        "out", [512, ROWS192["OUT"], W192], F32, isOutput=True)

    with tile.TileContext(nc) as tc:
        from contextlib import ExitStack
        with ExitStack() as ctx:
            pools = {}
            for name, kw in (
                ("const", dict(bufs=1)),
                ("oacc", dict(bufs=6)),
                ("pscr", dict(bufs=2)),
                ("psum", dict(bufs=8, space="PSUM")),
            ):
                pools[name] = ctx.enter_context(tc.tile_pool(name=name, **kw))
            # resident masks
            pools["m_u8"] = pools["const"].tile([128, ROWS192["A2"], W192], U8,
                                                name="m_u8_t", tag="m_u8")
            nc.sync.dma_start(pools["m_u8"][:], params["m_u8"][:])
            pools["rm192"] = pools["const"].tile([128, ROWS192["A2"]], BF16,
                                                 name="rm192_t", tag="rm192")
            nc.sync.dma_start(pools["rm192"][:], params["rm192"][:])

            pools["zt"] = pools["const"].tile([128, ZCHUNK], BF16,
                                               name="zt", tag="zt")
            nc.gpsimd.memset(pools["zt"][:], 0.0)
            _emit_l1(nc, tc, pools, bufs, params)
            _emit_l2(nc, tc, pools, bufs, params)
            with tc.tile_pool(name="wres", bufs=2) as p_wres:
                pools["wres"] = p_wres
                for lay in LAYERS:
                    if len(lay[4]) == 1:
                        _emit_dense192(nc, tc, pools, bufs, params, *lay)
                    else:
                        _emit_gather192(nc, tc, pools, bufs, params, *lay)
    nc.compile()
    return nc


# ---------------------------------------------------------------- host side

def _upsample_mask(m48):
    return np.repeat(np.repeat(m48, 4, axis=0), 4, axis=1)


def make_core_inputs(inputs, core):
    b, s = core // 4, core % 4
    r0, R0 = 48 * s, 96 * s
    x = np.asarray(inputs["batch"][b], np.float32)  # [3, 384, 384]

    canvas = np.zeros((3, CANVAS_ROWS, W384 + 2), np.float32)
    lo, hi = R0 - 42, R0 + 138
    clo, chi = max(lo, 0), min(hi, W384)
    canvas[:, clo - lo : chi - lo, 1 : 1 + W384] = x[:, clo:chi, :]

    x1col = np.empty((27, A1_ROWS, W384), np.float32)
    for t, (a, bb) in enumerate(TAPS):
        x1col[3 * t : 3 * t + 3] = canvas[:, 1 + a : 1 + a + A1_ROWS,
                                          1 + bb : 1 + bb + W384]

    m192 = _upsample_mask(np.asarray(inputs["pooling_mask"][b, 0]))  # [192,192]
    mbuf = np.zeros((ROWS192["A2"], W192), np.uint8)
    mlo, mhi = r0 - 20, r0 + 68
    cmlo, cmhi = max(mlo, 0), min(mhi, W192)
    mbuf[cmlo - mlo : cmhi - mlo] = m192[cmlo:cmhi].astype(np.uint8)

    rm384 = ((np.arange(CANVAS_ROWS) + R0 - 42 >= 0)
             & (np.arange(CANVAS_ROWS) + R0 - 42 < W384)).astype(np.float32)
    rm192 = ((np.arange(ROWS192["A2"]) + r0 - 20 >= 0)
             & (np.arange(ROWS192["A2"]) + r0 - 20 < W192)).astype(np.float32)

    im = {
        "x1col": x1col.astype(NPBF16),
        "m_u8": np.broadcast_to(mbuf, (128,) + mbuf.shape).copy(),
        "rm384": np.broadcast_to(rm384, (128, CANVAS_ROWS)).astype(NPBF16),
        "rm192": np.broadcast_to(rm192, (128, ROWS192["A2"])).astype(NPBF16),
    }
    w1 = np.asarray(inputs["w1"], np.float32)  # [64, 3, 3, 3]
    w1r = np.empty((27, 64), np.float32)
    for t, (a, bb) in enumerate(TAPS):
        w1r[3 * t : 3 * t + 3] = w1[:, :, a + 1, bb + 1].T
    im["w1"] = w1r.astype(NPBF16)
    for i in range(2, 11):
        w = np.asarray(inputs[f"w{i}"], np.float32)  # [O, I, 3, 3]
        im[f"w{i}"] = np.ascontiguousarray(
            w.transpose(1, 2, 3, 0).reshape(w.shape[1], 9, w.shape[0])
        ).astype(NPBF16)
    for i in range(1, 11):
        bv = np.asarray(inputs[f"b{i}"], np.float32)
        im[f"b{i}"] = np.ascontiguousarray(bv.reshape(-1, min(bv.size, 128)).T)
    return im


_NC_CACHE = []


def _get_program():
    if not _NC_CACHE:
        _NC_CACHE.append(build_program())
    return _NC_CACHE[0]


def kernel(**inputs):
    nc = _get_program()
    in_maps = [make_core_inputs(inputs, c) for c in range(8)]
    res = run_bass_kernel_spmd(nc, in_maps, list(range(8)))
    out = np.empty((2, 512, W192, W192), np.float32)
    for c in range(8):
        b, s = c // 4, c % 4
        out[b, :, 48 * s : 48 * s + 48, :] = res.results[c]["out"]
    return out


# revision 23
# speedup vs baseline: 1.7411x; 1.0256x over previous
"""Trainium2 Bass kernel for DenseFeatureExtractionModule (irregular-pooled VGG).

Sharding: 8 cores = 2 images x 4 row-strips of the 192-grid output (48 rows
each). Each core receives its input strip with enough halo rows to compute
all 10 conv layers locally (no inter-core communication). Out-of-image halo
rows are kept at zero through the layer stack by multiplying edge-band rows
with a per-core row-validity mask, which reproduces SAME-conv zero padding.

All activations + weights are bf16 (fp32 PSUM accumulation; rel err ~1.4e-2
vs the fp32 reference). Graph-conv layers (5-10) use a gather-select path:
per tap, ONE bf16 tensor_copy (4x DVE mode) of the dilation-1 window covering
all cin-tiles stacked along the free axis, plus one copy_predicated overlay
of the dilation-s window (center tap needs no select), then a single set of
N<=512 matmul chains — halving PE work vs dual-dilation. bf16 weights enable
PE fast-weight-load (LDWEIGHTS 107ns vs 195ns fp32). Edge-row masking and
irregular-pool max trees run on GpSimd/DVE off the critical path; DRAM
scratch buffers are zero-filled with large contiguous DMAs emitted at their
producing layer (column-strided pad writes clogged the DMA queues for ~0.5ms
at startup). Measured: 3.66ms on 8 cores (baseline 5.98-6.22ms).
"""

import numpy as np
import ml_dtypes

import concourse.bacc as bacc
import concourse.bass as bass
import concourse.mybir as mybir
import concourse.tile as tile
from concourse.bass_utils import run_bass_kernel_spmd

F32 = mybir.dt.float32
BF16 = mybir.dt.bfloat16
U8 = mybir.dt.uint8
RELU = mybir.ActivationFunctionType.Relu
MAX = mybir.AluOpType.max
NPBF16 = ml_dtypes.bfloat16

W192 = 192
W384 = 384
PAD = 4  # pad columns for all 192-grid buffers
WP = W192 + 2 * PAD  # 200
A1_WP = W384 + 2  # 386, pad 1

# buffer row counts (per core strips, incl. halo)
CANVAS_ROWS = 180  # batch strip on 384 grid (96 + 2*42)
A1_ROWS = 178
ROWS192 = {"A2": 88, "A3": 86, "A4": 84, "A5": 80, "A6": 76, "A7": 72,
           "A8": 64, "A9": 56, "OUT": 48}
HALO192 = {"A2": 20, "A3": 19, "A4": 18, "A5": 16, "A6": 14, "A7": 12,
           "A8": 8, "A9": 4, "OUT": 0}

_CHANS = [(3, 64), (64, 64), (64, 128), (128, 128), (128, 256),
          (256, 256), (256, 256), (256, 512), (512, 512), (512, 512)]

# (src, dst, Cin, Cout, dils, pool_s, widx) for layers 3..10
LAYERS = [
    ("A2", "A3", 64, 128, (1,), None, 3),
    ("A3", "A4", 128, 128, (1,), 2, 4),
    ("A4", "A5", 128, 256, (1, 2), None, 5),
    ("A5", "A6", 256, 256, (1, 2), None, 6),
    ("A6", "A7", 256, 256, (1, 2), 4, 7),
    ("A7", "A8", 256, 512, (1, 4), None, 8),
    ("A8", "A9", 512, 512, (1, 4), None, 9),
    ("A9", "OUT", 512, 512, (1, 4), None, 10),
]
RB = 8  # output rows per input-tile block (dense 192-grid layers)
GB = 4  # output rows per block in the gather path

TAPS = [(a, b) for a in (-1, 0, 1) for b in (-1, 0, 1)]


ZCHUNK = 4096  # free elems per partition per zero-fill DMA


def _emit_zero_buf(nc, pools, dram):
    """Zero one DRAM scratch buffer with large contiguous DMAs (the layer
    interior gets overwritten; only the pad columns need to be zero, but
    full-row zeroing is ~50x cheaper per byte than column-strided writes).
    Emitted just before the producing layer so the DMAs overlap its compute
    instead of clogging the queues at kernel start."""
    C, rows, wp = dram.shape
    total = rows * wp
    zt = pools["zt"]
    for ci in range((C + 127) // 128):
        p = min(128, C - ci * 128)
        flat = dram[ci * 128 : ci * 128 + p].rearrange("p r c -> p (r c)")
        for o in range(0, total, ZCHUNK):
            n = min(ZCHUNK, total - o)
            nc.sync.dma_start(flat[:, o : o + n], zt[:p, :n])


def _emit_l1(nc, tc, pools, bufs, params):
    """L1: 1x1 conv over im2col'd input (K=27, M=64), 384 grid."""
    x1, a1 = bufs["X1"], bufs["A1"]
    _emit_zero_buf(nc, pools, a1)
    wsb = pools["const"].tile([27, 64], BF16)
    nc.sync.dma_start(wsb[:], params["w1"][:])
    bsb = pools["const"].tile([64, 1], F32)
    nc.sync.dma_start(bsb[:], params["b1"][:])
    rm = pools["const"].tile([128, CANVAS_ROWS], BF16)
    nc.sync.dma_start(rm[:], params["rm384"][:])

    RB1 = 16
    with tc.tile_pool(name="in1", bufs=2) as p_in1, \
         tc.tile_pool(name="out1", bufs=4) as p_out1:
     for i0 in range(0, A1_ROWS, RB1):
        nr = min(RB1, A1_ROWS - i0)
        xt = p_in1.tile([27, RB1, W384], BF16, tag="x1t")
        nc.sync.dma_start(xt[:, :nr, :], x1[:, i0 : i0 + nr, :])
        for j in range(nr):
            i = i0 + j
            ps = pools["psum"].tile([64, W384], F32, tag="ps")
            nc.tensor.matmul(ps[:], wsb[:], xt[:, j, :], start=True, stop=True)
            ot = p_out1.tile([64, W384], BF16, tag="o1")
            nc.scalar.activation(ot[:], ps[:], RELU, bias=bsb[:])
            if i < 41 or i >= A1_ROWS - 41:
                nc.gpsimd.tensor_mul(
                    ot[:], ot[:],
                    rm[0:64, i + 1 : i + 2].broadcast_to([64, W384]))
            nc.sync.dma_start(a1[:, i, 1 : 1 + W384], ot[:])


def _emit_l2(nc, tc, pools, bufs, params):
    """L2 conv (64->64, 384 grid) fused with 2x2 maxpool -> A2 (192 grid).
    The a=-1 and a=0 tap rows are stacked into partitions 0-63 / 64-127 of
    one input tile so 6 of the 9 taps run as K=128 matmuls (6 mm/row not 9)."""
    a1, a2 = bufs["A1"], bufs["A2"]
    _emit_zero_buf(nc, pools, a2)
    wp2 = pools["const"].tile([128, 3, 64], BF16)
    nc.sync.dma_start(wp2[:], params["w2p"][:])
    ws2t = pools["const"].tile([128, 3, 64], BF16)
    nc.sync.dma_start(ws2t[64:128], params["w2s"][:])
    ws2 = ws2t[64:128]
    bsb = pools["const"].tile([64, 1], F32)
    nc.sync.dma_start(bsb[:], params["b2"][:])
    rm = pools["rm192"]

    QB = 8  # A2 rows per block
    with tc.tile_pool(name="in2", bufs=2) as p_in2, \
         tc.tile_pool(name="out2", bufs=4) as p_out2:
     for q0 in range(0, ROWS192["A2"], QB):
        nq = min(QB, ROWS192["A2"] - q0)
        lo = 2 * q0
        xt = p_in2.tile([128, 2 * QB + 2, A1_WP], BF16, tag="x2t")
        # partitions 0-63: a1 row lo+i;  64-127: a1 row lo+i+1
        nc.sync.dma_start(xt[0:64, : 2 * nq + 2, :],
                          a1[:, lo : lo + 2 * nq + 2, :])
        n2 = min(2 * nq + 2, A1_ROWS - lo - 1)
        nc.sync.dma_start(xt[64:128, :n2, :], a1[:, lo + 1 : lo + 1 + n2, :])
        for q in range(q0, q0 + nq):
            o2 = p_out2.tile([64, 2, W384], BF16, tag="o2")
            for r in range(2):
                R = 2 * (q - q0) + r
                ps = pools["psum"].tile([64, W384], F32, tag="ps")
                for bi in range(3):
                    nc.tensor.matmul(ps[:], wp2[:, bi, :],
                                     xt[:, R, bi : bi + W384],
                                     start=(bi == 0), stop=False)
                for bi in range(3):
                    nc.tensor.matmul(ps[:], ws2[:, bi, :],
                                     xt[64:128, R + 1, bi : bi + W384],
                                     start=False, stop=(bi == 2))
                nc.scalar.activation(o2[:, r, :], ps[:], RELU, bias=bsb[:])
            o2v = o2[:].rearrange("p r (c t) -> p r c t", t=2)
            cm = p_out2.tile([64, 2, W192], BF16, tag="cm2")
            nc.vector.tensor_tensor(cm[:, 0, :], o2v[:, 0, :, 0], o2v[:, 0, :, 1], MAX)
            nc.vector.tensor_tensor(cm[:, 1, :], o2v[:, 1, :, 0], o2v[:, 1, :, 1], MAX)
            mp = p_out2.tile([64, W192], BF16, tag="mp2")
            nc.vector.tensor_tensor(mp[:], cm[:, 0, :], cm[:, 1, :], MAX)
            if q < 20 or q >= ROWS192["A2"] - 20:
                nc.gpsimd.tensor_mul(
                    mp[:], mp[:], rm[0:64, q : q + 1].broadcast_to([64, W192]))
            nc.sync.dma_start(a2[:, q, PAD : PAD + W192], mp[:])


def _emit_dense192(nc, tc, pools, bufs, params, src, dst, cin, cout, dils,
                   pool_s, widx):
    """Dense 192-grid conv layer (single dilation), optional fused pool."""
    sdram, ddram = bufs[src], bufs[dst]
    if dst != "OUT":
        _emit_zero_buf(nc, pools, ddram)
    rows_out = ROWS192[dst]
    h_out = HALO192[dst]
    nci = (cin + 127) // 128
    nco = (cout + 127) // 128
    off = 20 - h_out
    rm = pools["rm192"]
    msb = pools["m_u8"]

    packed = cin == 64  # stack a=-1/a=0 tap rows into one K=128 tile
    if packed:
        wtp = pools["wres"].tile([128, 3, cout], BF16, name=f"w{widx}p",
                                 tag="wres_0")
        nc.sync.dma_start(wtp[:], params[f"w{widx}p"][:])
        wtst = pools["wres"].tile([128, 3, cout], BF16, name=f"w{widx}s",
                                  tag="wres_1")
        nc.sync.dma_start(wtst[64:128], params[f"w{widx}s"][:])
        wtss = wtst[64:128]
    else:
        wts = []
        for ci in range(nci):
            p = min(128, cin - ci * 128)
            wt = pools["wres"].tile([p, 9, cout], BF16, name=f"w{widx}_{ci}",
                                    tag=f"wres_{ci}")
            nc.sync.dma_start(wt[:], params[f"w{widx}"][ci * 128 : ci * 128 + p])
            wts.append(wt)
    bsb = pools["const"].tile([min(cout, 128), nco], F32, name=f"bsb{widx}")
    nc.sync.dma_start(bsb[:], params[f"b{widx}"][:])

    grp = pool_s if pool_s else 2  # rows per output tile group
    with tc.tile_pool(name=f"xin{widx}", bufs=2) as p_xin:
     for j0 in range(0, rows_out, RB):
        rb = min(RB, rows_out - j0)
        xts = []
        if packed:
            xt = p_xin.tile([128, RB + 2, WP], BF16, tag="xin0")
            nc.sync.dma_start(xt[0:64, : rb + 2, :],
                              sdram[:, j0 : j0 + rb + 2, :])
            n2 = min(rb + 2, ROWS192[src] - j0 - 1)
            nc.sync.dma_start(xt[64:128, :n2, :],
                              sdram[:, j0 + 1 : j0 + 1 + n2, :])
            xts.append(xt)
        else:
            for ci in range(nci):
                p = min(128, cin - ci * 128)
                xt = p_xin.tile([p, RB + 2, WP], BF16, tag=f"xin{ci}")
                nc.sync.dma_start(xt[:, : rb + 2, :],
                                  sdram[ci * 128 : ci * 128 + p, j0 : j0 + rb + 2, :])
                xts.append(xt)
        for co in range(nco):
            pco = min(128, cout - co * 128)
            for g0 in range(0, rb, grp):
                tg = pools["oacc"].tile([pco, grp, W192], BF16, tag="oacc")
                for rp in range(grp // 2):
                    j = j0 + g0 + rp * 2
                    ps = pools["psum"].tile([pco, 2 * W192], F32, tag="ps")
                    if packed:
                        R = g0 + rp * 2
                        for bi in range(3):
                            nc.tensor.matmul(
                                ps[:], wtp[:, bi, co * 128 : co * 128 + pco],
                                xts[0][:, R : R + 2,
                                       PAD - 1 + bi : PAD - 1 + bi + W192],
                                start=(bi == 0), stop=False)
                        for bi in range(3):
                            nc.tensor.matmul(
                                ps[:], wtss[:, bi, co * 128 : co * 128 + pco],
                                xts[0][64:128, R + 1 : R + 3,
                                       PAD - 1 + bi : PAD - 1 + bi + W192],
                                start=False, stop=(bi == 2))
                    else:
                      for ci in range(nci):
                        for ti, (a, b) in enumerate(TAPS):
                            rhs = xts[ci][:, g0 + rp * 2 + 1 + a :
                                          g0 + rp * 2 + 1 + a + 2,
                                          PAD + b : PAD + b + W192]
                            nc.tensor.matmul(
                                ps[:],
                                wts[ci][:, ti, co * 128 : co * 128 + pco],
                                rhs,
                                start=(ci == 0 and ti == 0),
                                stop=(ci == nci - 1 and ti == 8))
                    t1 = tg[:, rp * 2 : rp * 2 + 2, :]
                    psv = ps[:].rearrange("p (r w) -> p r w", w=W192)
                    nc.scalar.activation(t1, psv, RELU, bias=bsb[:pco, co : co + 1])
                    if j < h_out or j + 2 > rows_out - h_out:
                        nc.gpsimd.tensor_mul(
                            t1, t1,
                            rm[:pco, off + j : off + j + 2].unsqueeze(-1)
                            .broadcast_to([pco, 2, W192]))
                j = j0 + g0
                if pool_s == 2:
                    tv = tg[:].rearrange("p r (c t) -> p r c t", t=2)
                    cm = pools["pscr"].tile([pco, 2, W192 // 2], BF16, tag="pcm")
                    nc.vector.tensor_tensor(cm[:], tv[:, :, :, 0], tv[:, :, :, 1], MAX)
                    bm = pools["pscr"].tile([pco, W192 // 2], BF16, tag="pbm")
                    nc.vector.tensor_tensor(bm[:], cm[:, 0, :], cm[:, 1, :], MAX)
                    rep = pools["pscr"].tile([pco, 2, W192], BF16, tag="prep")
                    nc.vector.tensor_copy(
                        rep[:], bm[:].unsqueeze(1).unsqueeze(-1)
                        .broadcast_to([pco, 2, W192 // 2, 2]))
                    nc.vector.copy_predicated(
                        tg[:], msb[:pco, off + j : off + j + 2, :], rep[:])
                nc.sync.dma_start(
                    ddram[co * 128 : co * 128 + pco, j : j + grp,
                          PAD : PAD + W192], tg[:])


def _emit_gather192(nc, tc, pools, bufs, params, src, dst, cin, cout, dils,
                    pool_s, widx):
    """Graph-conv layer via gather-select: one bf16 copy (4x DVE mode) + one
    copy_predicated overlay per tap covering ALL cin-tiles (stacked along the
    free axis), then N<=512 matmul chains. Optional fused s=4 pool."""
    sdram, ddram = bufs[src], bufs[dst]
    if dst != "OUT":
        _emit_zero_buf(nc, pools, ddram)
    rows_out = ROWS192[dst]
    h_out = HALO192[dst]
    s = dils[1]
    dm = s
    nci = (cin + 127) // 128
    nco = (cout + 127) // 128
    off = 20 - h_out
    act_dt = F32 if dst == "OUT" else BF16
    rm = pools["rm192"]
    msb = pools["m_u8"]
    dst_c0 = 0 if dst == "OUT" else PAD
    GBl = 8 if nco <= 2 else 4  # psum banks: nco * ceil(GBl*192/512) <= 8
    S = GBl + 2 * dm  # xin rows per cin-tile slot

    wts = []
    for ci in range(nci):
        wt = pools["wres"].tile([128, 9, cout], BF16, name=f"w{widx}_{ci}",
                                tag=f"wres_{ci}")
        nc.sync.dma_start(wt[:], params[f"w{widx}"][ci * 128 : ci * 128 + 128])
        wts.append(wt)
    bsb = pools["const"].tile([min(cout, 128), nco], F32, name=f"bsb{widx}")
    nc.sync.dma_start(bsb[:], params[f"b{widx}"][:])

    with tc.tile_pool(name=f"xin{widx}", bufs=2) as p_xin, \
         tc.tile_pool(name=f"gp{widx}", bufs=6) as p_g:
        for j0 in range(0, rows_out, GBl):
            gb = min(GBl, rows_out - j0)
            ncol = gb * W192
            bounds = [(k * 512, min(ncol, (k + 1) * 512))
                      for k in range((ncol + 511) // 512)]
            xt = p_xin.tile([128, nci * S, WP], BF16, tag="xin")
            for ci in range(nci):
                nc.sync.dma_start(
                    xt[:, ci * S : ci * S + gb + 2 * dm, :],
                    sdram[ci * 128 : ci * 128 + 128, j0 : j0 + gb + 2 * dm, :])
            xv = xt[:].rearrange("p (n r) c -> p n r c", n=nci)
            pss = [[pools["psum"].tile([128, hi - lo], F32, tag="ps",
                                       name=f"ps{widx}_{j0}_{co}_{k}")
                    for k, (lo, hi) in enumerate(bounds)] for co in range(nco)]
            for ti, (a, b) in enumerate(TAPS):
                g = p_g.tile([128, nci, GBl, W192], BF16, tag="g3")
                nc.vector.tensor_copy(
                    g[:, :, :gb, :], xv[:, :, dm + a : dm + a + gb,
                                        PAD + b : PAD + b + W192])
                if not (a == 0 and b == 0):
                    nc.vector.copy_predicated(
                        g[:, :, :gb, :],
                        msb[:, off + j0 : off + j0 + gb, :].unsqueeze(1)
                           .broadcast_to([128, nci, gb, W192]),
                        xv[:, :, dm + a * s : dm + a * s + gb,
                           PAD + b * s : PAD + b * s + W192])
                gf = g[:].rearrange("p n r w -> p n (r w)")
                for ci in range(nci):
                    for co in range(nco):
                        pco = min(128, cout - co * 128)
                        for k, (lo, hi) in enumerate(bounds):
                            nc.tensor.matmul(
                                pss[co][k][:pco, :],
                                wts[ci][:, ti, co * 128 : co * 128 + pco],
                                gf[:, ci, lo:hi],
                                start=(ti == 0 and ci == 0),
                                stop=(ti == 8 and ci == nci - 1))
            for co in range(nco):
                pco = min(128, cout - co * 128)
                tg = pools["oacc"].tile([pco, GBl, W192], act_dt, tag="oacc")
                tgf = tg[:].rearrange("p r w -> p (r w)")
                for k, (lo, hi) in enumerate(bounds):
                    nc.scalar.activation(tgf[:, lo:hi], pss[co][k][:pco, :],
                                         RELU, bias=bsb[:pco, co : co + 1])
                if j0 < h_out or j0 + gb > rows_out - h_out:
                    nc.gpsimd.tensor_mul(
                        tg[:, :gb, :], tg[:, :gb, :],
                        rm[:pco, off + j0 : off + j0 + gb].unsqueeze(-1)
                        .broadcast_to([pco, gb, W192]))
                if pool_s == 4:
                    for q0 in range(0, gb, 4):
                        tq = tg[:, q0 : q0 + 4, :]
                        tv = tq.rearrange("p r (c t) -> p r c t", t=4)
                        c1 = pools["pscr"].tile([pco, 4, W192 // 4], BF16, tag="pc1")
                        c2 = pools["pscr"].tile([pco, 4, W192 // 4], BF16, tag="pc2")
                        nc.vector.tensor_tensor(c1[:], tv[:, :, :, 0], tv[:, :, :, 1], MAX)
                        nc.vector.tensor_tensor(c2[:], tv[:, :, :, 2], tv[:, :, :, 3], MAX)
                        nc.vector.tensor_tensor(c1[:], c1[:], c2[:], MAX)
                        r1 = pools["pscr"].tile([pco, W192 // 4], BF16, tag="pr1")
                        r2 = pools["pscr"].tile([pco, W192 // 4], BF16, tag="pr2")
                        nc.vector.tensor_tensor(r1[:], c1[:, 0, :], c1[:, 1, :], MAX)
                        nc.vector.tensor_tensor(r2[:], c1[:, 2, :], c1[:, 3, :], MAX)
                        nc.vector.tensor_tensor(r1[:], r1[:], r2[:], MAX)
                        rep = pools["pscr"].tile([pco, 4, W192], BF16, tag="prep4")
                        nc.vector.tensor_copy(
                            rep[:], r1[:].unsqueeze(1).unsqueeze(-1)
                            .broadcast_to([pco, 4, W192 // 4, 4]))
                        nc.vector.copy_predicated(
                            tq, msb[:pco, off + j0 + q0 : off + j0 + q0 + 4, :],
                            rep[:])
                nc.sync.dma_start(
                    ddram[co * 128 : co * 128 + pco, j0 : j0 + gb,
                          dst_c0 : dst_c0 + W192], tg[:, :gb, :])


def build_program():
    nc = bacc.Bacc()
    params = {}
    params["x1col"] = nc.declare_dram_parameter(
        "x1col", [27, A1_ROWS, W384], BF16, isOutput=False)
    params["w1"] = nc.declare_dram_parameter("w1", [27, 64], BF16, isOutput=False)
    for i, (ci, co) in enumerate(_CHANS):
        if i + 1 in (2, 3):
            params[f"w{i + 1}p"] = nc.declare_dram_parameter(
                f"w{i + 1}p", [128, 3, co], BF16, isOutput=False)
            params[f"w{i + 1}s"] = nc.declare_dram_parameter(
                f"w{i + 1}s", [64, 3, co], BF16, isOutput=False)
        elif i > 0:
            params[f"w{i + 1}"] = nc.declare_dram_parameter(
                f"w{i + 1}", [ci, 9, co], BF16, isOutput=False)
        params[f"b{i + 1}"] = nc.declare_dram_parameter(
            f"b{i + 1}", [min(co, 128), (co + 127) // 128], F32, isOutput=False)
    params["m_u8"] = nc.declare_dram_parameter(
        "m_u8", [128, ROWS192["A2"], W192], U8, isOutput=False)
    params["rm384"] = nc.declare_dram_parameter(
        "rm384", [128, CANVAS_ROWS], BF16, isOutput=False)
    params["rm192"] = nc.declare_dram_parameter(
        "rm192", [128, ROWS192["A2"]], BF16, isOutput=False)

    bufs = {"X1": params["x1col"]}
    bufs["A1"] = nc.dram_tensor("A1", [64, A1_ROWS, A1_WP], BF16)
    for name, cc in (("A2", 64), ("A3", 128), ("A4", 128), ("A5", 256),
                     ("A6", 256), ("A7", 256), ("A8", 512), ("A9", 512)):
        bufs[name] = nc.dram_tensor(name, [cc, ROWS192[name], WP], BF16)
    bufs["OUT"] = nc.declare_dram_parameter(
        "out", [512, ROWS192["OUT"], W192], F32, isOutput=True)

    with tile.TileContext(nc) as tc:
        from contextlib import ExitStack
        with ExitStack() as ctx:
            pools = {}
            for name, kw in (
                ("const", dict(bufs=1)),
                ("oacc", dict(bufs=6)),
                ("pscr", dict(bufs=2)),
                ("psum", dict(bufs=8, space="PSUM")),
            ):
                pools[name] = ctx.enter_context(tc.tile_pool(name=name, **kw))
            # resident masks
            pools["m_u8"] = pools["const"].tile([128, ROWS192["A2"], W192], U8,
                                                name="m_u8_t", tag="m_u8")
            nc.sync.dma_start(pools["m_u8"][:], params["m_u8"][:])
            pools["rm192"] = pools["const"].tile([128, ROWS192["A2"]], BF16,
                                                 name="rm192_t", tag="rm192")
            nc.sync.dma_start(pools["rm192"][:], params["rm192"][:])

            pools["zt"] = pools["const"].tile([128, ZCHUNK], BF16,
                                               name="zt", tag="zt")
            nc.gpsimd.memset(pools["zt"][:], 0.0)
            _emit_l1(nc, tc, pools, bufs, params)
            _emit_l2(nc, tc, pools, bufs, params)
            with tc.tile_pool(name="wres", bufs=2) as p_wres:
                pools["wres"] = p_wres
                for lay in LAYERS:
                    if len(lay[4]) == 1:
                        _emit_dense192(nc, tc, pools, bufs, params, *lay)
                    else:
                        _emit_gather192(nc, tc, pools, bufs, params, *lay)
    nc.compile()
    return nc


# ---------------------------------------------------------------- host side

def _upsample_mask(m48):
    return np.repeat(np.repeat(m48, 4, axis=0), 4, axis=1)


def make_core_inputs(inputs, core):
    b, s = core // 4, core % 4
    r0, R0 = 48 * s, 96 * s
    x = np.asarray(inputs["batch"][b], np.float32)  # [3, 384, 384]

    canvas = np.zeros((3, CANVAS_ROWS, W384 + 2), np.float32)
    lo, hi = R0 - 42, R0 + 138
    clo, chi = max(lo, 0), min(hi, W384)
    canvas[:, clo - lo : chi - lo, 1 : 1 + W384] = x[:, clo:chi, :]

    x1col = np.empty((27, A1_ROWS, W384), np.float32)
    for t, (a, bb) in enumerate(TAPS):
        x1col[3 * t : 3 * t + 3] = canvas[:, 1 + a : 1 + a + A1_ROWS,
                                          1 + bb : 1 + bb + W384]

    m192 = _upsample_mask(np.asarray(inputs["pooling_mask"][b, 0]))  # [192,192]
    mbuf = np.zeros((ROWS192["A2"], W192), np.uint8)
    mlo, mhi = r0 - 20, r0 + 68
    cmlo, cmhi = max(mlo, 0), min(mhi, W192)
    mbuf[cmlo - mlo : cmhi - mlo] = m192[cmlo:cmhi].astype(np.uint8)

    rm384 = ((np.arange(CANVAS_ROWS) + R0 - 42 >= 0)
             & (np.arange(CANVAS_ROWS) + R0 - 42 < W384)).astype(np.float32)
    rm192 = ((np.arange(ROWS192["A2"]) + r0 - 20 >= 0)
             & (np.arange(ROWS192["A2"]) + r0 - 20 < W192)).astype(np.float32)

    im = {
        "x1col": x1col.astype(NPBF16),
        "m_u8": np.broadcast_to(mbuf, (128,) + mbuf.shape).copy(),
        "rm384": np.broadcast_to(rm384, (128, CANVAS_ROWS)).astype(NPBF16),
        "rm192": np.broadcast_to(rm192, (128, ROWS192["A2"])).astype(NPBF16),
    }
    w1 = np.asarray(inputs["w1"], np.float32)  # [64, 3, 3, 3]
    w1r = np.empty((27, 64), np.float32)
    for t, (a, bb) in enumerate(TAPS):
        w1r[3 * t : 3 * t + 3] = w1[:, :, a + 1, bb + 1].T
    im["w1"] = w1r.astype(NPBF16)
    for i in range(2, 11):
        w = np.asarray(inputs[f"w{i}"], np.float32)  # [O, I, 3, 3]
        wr = np.ascontiguousarray(
            w.transpose(1, 2, 3, 0).reshape(w.shape[1], 9, w.shape[0]))
        if i in (2, 3):
            co = wr.shape[2]
            wp = np.empty((128, 3, co), np.float32)
            wp[0:64] = wr[:, 0:3]    # taps (a=-1, b)
            wp[64:128] = wr[:, 3:6]  # taps (a=0, b)
            im[f"w{i}p"] = wp.astype(NPBF16)
            im[f"w{i}s"] = np.ascontiguousarray(wr[:, 6:9]).astype(NPBF16)
        else:
            im[f"w{i}"] = wr.astype(NPBF16)
    for i in range(1, 11):
        bv = np.asarray(inputs[f"b{i}"], np.float32)
        im[f"b{i}"] = np.ascontiguousarray(bv.reshape(-1, min(bv.size, 128)).T)
    return im


_NC_CACHE = []


def _get_program():
    if not _NC_CACHE:
        _NC_CACHE.append(build_program())
    return _NC_CACHE[0]


def kernel(**inputs):
    nc = _get_program()
    in_maps = [make_core_inputs(inputs, c) for c in range(8)]
    res = run_bass_kernel_spmd(nc, in_maps, list(range(8)))
    out = np.empty((2, 512, W192, W192), np.float32)
    for c in range(8):
        b, s = c // 4, c % 4
        out[b, :, 48 * s : 48 * s + 48, :] = res.results[c]["out"]
    return out
